# revision 3
# baseline (speedup 1.0000x reference)
"""Two-layer GAT (PyG GATConv semantics) on 8 Trainium2 NeuronCores — v2.

Strategy (graph/data parallel): dst nodes range-sharded across 8 cores;
every core redundantly computes the full layer-1 node-feature table
(h = x @ W1ext, with attention projections es/ed folded in as extra
columns); edges sorted by dst and bucketed per 128-dst tile.

v2 batches all per-edge-group elementwise work into per-tile ops
(the v1 kernel spent 1.5 ms in ~5000 tiny DVE instructions), folds the
dst-row fetch into the main gather, loads each tile's one-hot masks in
a single DMA, and pipelines the layer-2 table AllGather in 5-tile
chunks under the layer-1 compute.

Row table layouts (bf16 elements):
  L1 row (640): [h0 h1 (256) | 1 1 (2) | h2 h3 (256) | 1 1 (2) |
                 es f32 x4 (516:524) | ed f32 x4 (524:532) | pad]
  The ones columns ride along in the weighted-feature matmul so a single
  pair of one-hot matmuls yields both the attention-weighted sums and the
  softmax denominators.
  L2 row (128): [h2w2 (64) | 1 (1) | pad (1) | es2 f32 (66:68) |
                 ed2 f32 (68:70) | pad]
  t2full rows are permuted: node (c,t,p) with b=t//5, q=t%5 lives at row
  b*5120 + c*640 + q*128 + p, so each 5-tile AllGather chunk lands
  contiguously.
"""

import sys

for _p in ("/opt/trn_rl_repo",):
    if _p not in sys.path:
        sys.path.insert(0, _p)

import numpy as np
import ml_dtypes

import concourse.bacc as bacc
import concourse.bass as bass
import concourse.mybir as mybir
import concourse.tile as tile
from concourse import library_config
from concourse._compat import axon_active
from concourse.bass_utils import run_bass_kernel_spmd
from concourse.masks import make_identity

BF16 = ml_dtypes.bfloat16
F32 = mybir.dt.float32
BF = mybir.dt.bfloat16
P = 128
NCORES = 8
CBOUNDS = [0, 5, 10, 15, 20, 24, 25]  # AllGather chunk tile boundaries
NQUEUES = 2  # SWDGE queues for gather desc-gen


class GATConfig:
    def __init__(self, n, in_ch, hid, heads, out_ch, neg_slope, ng):
        self.N = n
        self.NPAD = -(-n // (P * NCORES)) * (P * NCORES)
        self.SHARD = self.NPAD // NCORES
        self.T = self.SHARD // P              # dst tiles per core
        self.NT = self.NPAD // P              # node tiles (phase A)
        self.IN_CH = in_ch
        self.KIN = in_ch // P
        self.HID = hid                        # 128
        self.HEADS = heads                    # 4
        self.OUT_CH = out_ch                  # 64
        self.NEG = neg_slope
        self.NG = ng                          # edge groups per dst tile
        self.H1 = heads * hid                 # 512
        # L1 row layout (see module docstring)
        self.R1_ES = 516
        self.R1_ED = 524
        self.ROW1 = 640
        # L2 row layout
        self.R2_ES = out_ch + 2               # 66
        self.R2_ED = out_ch + 4               # 68
        self.ROW2 = 128
        self.KH1 = self.H1 // P               # 4
        assert heads == 4 and hid == 128 and in_ch % P == 0


def _wrap_idx(flat):
    """int16 flat index list -> [128, len/16] wrapped layout for dma_gather."""
    n = len(flat)
    assert n % 16 == 0
    w = np.asarray(flat, np.int16).reshape(n // 16, 16).T  # [16, n/16]
    return np.tile(w, (8, 1))                              # [128, n/16]


def _t2row(node, T):
    """t2full row index for a global node id (chunked-allgather layout)."""
    node = np.asarray(node)
    c = node // (T * P)
    r = node % (T * P)
    t = r // P
    p = r % P
    cb = np.asarray(CBOUNDS)
    k = np.searchsorted(cb, t, side="right") - 1
    w = (cb[k + 1] - cb[k]) * P
    return cb[k] * P * NCORES + c * w + (t - cb[k]) * P + p


def host_prep(cfg, x, edge_index, W1, a_src1, a_dst1, b1, W2, a_src2, a_dst2, b2):
    """Build all per-core input arrays. Returns (in_maps, meta)."""
    N, NPAD = cfg.N, cfg.NPAD
    H, C, OC = cfg.HEADS, cfg.HID, cfg.OUT_CH
    T, NG = cfg.T, cfg.NG

    # --- weights: W1 natural + es/ed projection columns appended --------
    W1 = np.asarray(W1, np.float32)
    W2 = np.asarray(W2, np.float32)
    a_src1 = np.asarray(a_src1, np.float32)
    a_dst1 = np.asarray(a_dst1, np.float32)
    a_src2 = np.asarray(a_src2, np.float32)
    a_dst2 = np.asarray(a_dst2, np.float32)
    w1ext = np.zeros((cfg.IN_CH, cfg.H1 + 8), np.float32)
    w1ext[:, : cfg.H1] = W1
    for h in range(H):
        w1ext[:, cfg.H1 + h] = W1[:, h * C : (h + 1) * C] @ a_src1[h]
        w1ext[:, cfg.H1 + 4 + h] = W1[:, h * C : (h + 1) * C] @ a_dst1[h]
    w1eh = np.ascontiguousarray(
        w1ext.reshape(cfg.KIN, P, cfg.H1 + 8).transpose(1, 0, 2)
    ).astype(BF16)                                        # [128, KIN, 520]

    w2ext = np.zeros((cfg.H1, OC + 2), np.float32)
    w2ext[:, :OC] = W2
    w2ext[:, OC] = W2 @ a_src2[0]
    w2ext[:, OC + 1] = W2 @ a_dst2[0]
    w2eh = np.ascontiguousarray(
        w2ext.reshape(cfg.KH1, P, OC + 2).transpose(1, 0, 2)
    ).astype(BF16)                                        # [128, KH1, 66]

    # --- x, transposed+tiled for lhsT ------------------------------------
    xp = np.zeros((NPAD, cfg.IN_CH), np.float32)
    xp[:N] = np.asarray(x, np.float32)
    xth = np.ascontiguousarray(
        xp.reshape(cfg.NT, P, cfg.KIN, P).transpose(3, 0, 2, 1)
    ).reshape(P, cfg.NT * cfg.KIN * P).astype(BF16)

    # --- edges ------------------------------------------------------------
    ei = np.asarray(edge_index, np.int64)
    loop = np.arange(N, dtype=np.int64)
    src = np.concatenate([ei[0], loop])
    dst = np.concatenate([ei[1], loop])
    order = np.argsort(dst, kind="stable")
    src_s = src[order].astype(np.int32)
    dst_s = dst[order].astype(np.int32)
    gtiles = cfg.NT
    counts = np.bincount(dst_s // P, minlength=gtiles)
    ng = int(np.ceil(counts.max() / P)) if counts.max() else 1
    assert ng <= NG, f"data needs NG={ng} > configured {NG}"
    ET = NG * P
    starts = np.concatenate([[0], np.cumsum(counts)])

    t2map = _t2row(np.arange(NPAD), T)                    # node -> t2full row

    gidx = np.zeros((NCORES, T, P, (NG + 1) * 8), np.int16)
    gidx2 = np.zeros((NCORES, T, P, (NG + 1) * 8), np.int16)
    maskw = np.zeros((NCORES, P, T, NG, 2 * P), BF16)
    eye = np.arange(P, dtype=np.int32)
    for gt in range(gtiles):
        c, t = divmod(gt, T)
        lo, hi = starts[gt], starts[gt + 1]
        k = hi - lo
        idx = np.zeros(ET + P, np.int32)
        idx[:k] = src_s[lo:hi]
        idx[ET : ET + P] = gt * P + eye                   # own dst rows
        dl = np.full(ET, -1, np.int32)
        dl[:k] = dst_s[lo:hi] - gt * P
        # pad dst nodes get a dummy self-edge so their softmax denom is > 0
        pads = eye[gt * P + eye >= N]
        assert k + len(pads) <= ET
        idx[k : k + len(pads)] = gt * P + pads
        dl[k : k + len(pads)] = pads
        gidx[c, t] = _wrap_idx(idx.astype(np.int16))
        gidx2[c, t, :, : NG * 8] = _wrap_idx(t2map[idx[:ET]].astype(np.int16))
        m = dl.reshape(NG, P, 1) == eye.reshape(1, 1, P)  # [NG, e, d]
        maskw[c, :, t, :, :P] = m.transpose(1, 0, 2)      # [e, NG, d]
        maskw[c, :, t, :, P:] = m.transpose(2, 0, 1)      # [d, NG, e]

    in_maps = []
    for c in range(NCORES):
        in_maps.append(
            {
                "xth": xth,
                "w1eh": w1eh,
                "w2eh": w2eh,
                "gidx": gidx[c].reshape(T * P, (NG + 1) * 8),
                "gidx2": gidx2[c].reshape(T * P, (NG + 1) * 8),
                "maskw": maskw[c].reshape(P, T * NG * 2 * P),
            }
        )
    meta = {
        "b1_nonzero": bool(np.any(np.asarray(b1))),
        "b2_nonzero": bool(np.any(np.asarray(b2))),
        "b1": np.asarray(b1, np.float32),
        "b2": np.asarray(b2, np.float32),
    }
    return in_maps, meta


def build_program(cfg, meta, phases="ABCD", chunked_cc=True):
    under_axon = axon_active()
    nc = bacc.Bacc(
        "TRN2",
        target_bir_lowering=False,
        debug=not under_axon,
        num_devices=NCORES,
        dynamic_dma_scratch_size=65536,
        num_swdge_queues=NQUEUES,
    )
    H, C, OC, NG, T = cfg.HEADS, cfg.HID, cfg.OUT_CH, cfg.NG, cfg.T
    H1, KIN, KH1 = cfg.H1, cfg.KIN, cfg.KH1
    ROW1, ROW2 = cfg.ROW1, cfg.ROW2
    ACHUNK = 25                               # phase-A node tiles per x chunk

    xth_d = nc.dram_tensor("xth", [P, cfg.NT * KIN * P], BF, kind="ExternalInput")
    w1eh_d = nc.dram_tensor("w1eh", [P, KIN, H1 + 8], BF, kind="ExternalInput")
    w2eh_d = nc.dram_tensor("w2eh", [P, KH1, OC + 2], BF, kind="ExternalInput")
    gidx_d = nc.dram_tensor("gidx", [T * P, (NG + 1) * 8], mybir.dt.int16,
                            kind="ExternalInput")
    gidx2_d = nc.dram_tensor("gidx2", [T * P, (NG + 1) * 8], mybir.dt.int16,
                             kind="ExternalInput")
    maskw_d = nc.dram_tensor("maskw", [P, T * NG * 2 * P], BF,
                             kind="ExternalInput")
    out_d = nc.dram_tensor("out", [cfg.SHARD, OC], F32, kind="ExternalOutput")

    table1 = nc.dram_tensor("table1", [cfg.NPAD, ROW1], BF)
    t2shard = nc.dram_tensor("t2shard", [cfg.SHARD, ROW2], BF)
    t2full = nc.dram_tensor("t2full", [cfg.NPAD, ROW2], BF, addr_space="Shared")

    if meta["b1_nonzero"]:
        b1_d = nc.dram_tensor("b1", [P, H1], F32, kind="ExternalInput")
    if meta["b2_nonzero"]:
        b2_d = nc.dram_tensor("b2", [P, OC], F32, kind="ExternalInput")

    with tile.TileContext(nc) as tc:
        nc.gpsimd.load_library(library_config.mlp)

        with tc.tile_pool(name="persist", bufs=1) as pp:
            w1eh = pp.tile([P, KIN, H1 + 8], BF)
            nc.sync.dma_start(out=w1eh[:], in_=w1eh_d[:])
            w2eh = pp.tile([P, KH1, OC + 2], BF)
            nc.sync.dma_start(out=w2eh[:], in_=w2eh_d[:])
            gidx = pp.tile([P, T, (NG + 1) * 8], mybir.dt.int16)
            nc.sync.dma_start(
                out=gidx[:], in_=gidx_d[:].rearrange("(t p) s -> p t s", p=P)
            )
            gidx2 = pp.tile([P, T, (NG + 1) * 8], mybir.dt.int16)
            nc.sync.dma_start(
                out=gidx2[:], in_=gidx2_d[:].rearrange("(t p) s -> p t s", p=P)
            )
            ident = pp.tile([P, P], BF)
            make_identity(nc, ident[:])
            if meta["b1_nonzero"]:
                b1_sb = pp.tile([P, H1], F32)
                nc.sync.dma_start(out=b1_sb[:], in_=b1_d[:])
            if meta["b2_nonzero"]:
                b2_sb = pp.tile([P, OC], F32)
                nc.sync.dma_start(out=b2_sb[:], in_=b2_d[:])

            # persistent stage buffers with ones/pad pre-set
            NSTAGE = 3
            stages = []
            for i in range(NSTAGE):
                s = pp.tile([P, 2, ROW1], BF, name=f"stage{i}")
                nc.vector.memset(s[:, :, 256:258], 1.0)
                nc.vector.memset(s[:, :, 514:516], 1.0)
                nc.vector.memset(s[:, :, 532:ROW1], 0.0)
                stages.append(s)
            stages2 = []
            for i in range(NSTAGE):
                s = pp.tile([P, ROW2], BF, name=f"stage2_{i}")
                nc.vector.memset(s[:, OC : OC + 2], 0.0)
                nc.vector.memset(s[:, OC : OC + 1], 1.0)
                nc.vector.memset(s[:, cfg.R2_ED + 2 :], 0.0)
                stages2.append(s)

            # ---------------- Phase A: h table ---------------------------
            if "A" not in phases:
                raise ValueError("phase A required")
            with (
                tc.tile_pool(name="xc_pool", bufs=2) as xcp,
                tc.tile_pool(name="pa_ps", bufs=2, space="PSUM") as pa_ps,
            ):
                for c0 in range(0, cfg.NT, ACHUNK):
                    xc = xcp.tile([P, ACHUNK * KIN * P], BF, tag="xc")
                    nc.sync.dma_start(
                        out=xc[:],
                        in_=xth_d[:, c0 * KIN * P : (c0 + ACHUNK) * KIN * P],
                    )
                    for i in range(ACHUNK):
                        nt = c0 + i
                        ps01 = pa_ps.tile([P, 256], F32, tag="ps01")
                        ps23 = pa_ps.tile([P, 264], F32, tag="ps23")
                        for j in range(KIN):
                            lhs = xc[:, (i * KIN + j) * P : (i * KIN + j + 1) * P]
                            nc.tensor.matmul(
                                out=ps01[:], lhsT=lhs, rhs=w1eh[:, j, 0:256],
                                start=(j == 0), stop=(j == KIN - 1),
                            )
                            nc.tensor.matmul(
                                out=ps23[:], lhsT=lhs, rhs=w1eh[:, j, 256:520],
                                start=(j == 0), stop=(j == KIN - 1),
                            )
                        stage = stages[(nt // 2) % NSTAGE]
                        half = nt % 2
                        nc.vector.tensor_copy(
                            out=stage[:, half, 0:256], in_=ps01[:]
                        )
                        nc.vector.tensor_copy(
                            out=stage[:, half, 258:514], in_=ps23[:, 0:256]
                        )
                        nc.scalar.activation(
                            out=stage[:, half, 516:532].bitcast(F32),
                            in_=ps23[:, 256:264],
                            func=mybir.ActivationFunctionType.Copy,
                        )
                        if half == 1:
                            nc.sync.dma_start(
                                out=table1[(nt - 1) * P : (nt + 1) * P, :].rearrange(
                                    "(t p) r -> p t r", p=P
                                ),
                                in_=stage[:],
                            )

            # ---------------- Phases B/C/D --------------------------------
            with (
                tc.tile_pool(name="mask_sb", bufs=2) as msb,
                tc.tile_pool(name="small_sb", bufs=2) as ssb,
            ):
              with (
                tc.tile_pool(name="hg_sb", bufs=2) as hgp,
                tc.tile_pool(name="hg2_sb", bufs=3) as hgp2,
                tc.tile_pool(name="ph_sb", bufs=2) as php,
                tc.tile_pool(name="elu_sb", bufs=1) as elup,
                tc.tile_pool(name="acc_ps", bufs=2, space="PSUM") as aps,
                tc.tile_pool(name="tp_ps", bufs=1, space="PSUM") as tps,
              ):
                # ---- layer 1 aggregation + table2 rows + chunked gather --
                for t in range(T if "B" in phases else 0):
                    hg = hgp.tile([P, NG + 1, ROW1], BF, tag="hg")
                    for ci, c0 in enumerate(range(0, NG + 1, 8)):
                        gch = min(8, NG + 1 - c0)
                        nc.gpsimd.dma_gather(
                            out_ap=hg[:, c0 : c0 + gch, :],
                            in_ap=table1[:],
                            idxs_ap=gidx[:, t, c0 * 8 : (c0 + gch) * 8],
                            num_idxs=gch * P,
                            num_idxs_reg=gch * P,
                            elem_size=ROW1,
                            queue_num=(t * 3 + ci) % NQUEUES,
                        )
                    mp = msb.tile([P, NG, 2 * P], BF, tag="mp")
                    nc.sync.dma_start(
                        out=mp[:],
                        in_=maskw_d[:, t * NG * 2 * P : (t + 1) * NG * 2 * P],
                    )
                    # ed for this tile's dsts (from the appended dst rows)
                    edbf = ssb.tile([P, H], BF, tag="edbf")
                    nc.scalar.activation(
                        out=edbf[:],
                        in_=hg[:, NG, cfg.R1_ED : cfg.R1_ED + 8].bitcast(F32),
                        func=mybir.ActivationFunctionType.Copy,
                    )
                    # ed scattered to edge slots: one psum, NG matmuls
                    ep = aps.tile([P, NG * H], F32, tag="ep")
                    for g in range(NG):
                        nc.tensor.matmul(
                            out=ep[:, g * H : (g + 1) * H],
                            lhsT=mp[:, g, P : 2 * P],
                            rhs=edbf[:],
                            start=True, stop=True,
                        )
                    # batched logits chain
                    elog = ssb.tile([P, NG, H], F32, tag="elog")
                    nc.vector.tensor_tensor(
                        out=elog[:],
                        in0=hg[:, 0:NG, cfg.R1_ES : cfg.R1_ES + 8].bitcast(F32),
                        in1=ep[:].rearrange("p (g h) -> p g h", h=H),
                        op=mybir.AluOpType.add,
                    )
                    e1 = ssb.tile([P, NG, H], BF, tag="e1")
                    nc.scalar.activation(
                        out=e1[:], in_=elog[:],
                        func=mybir.ActivationFunctionType.Exp,
                    )
                    e2 = ssb.tile([P, NG, H], BF, tag="e2")
                    nc.scalar.activation(
                        out=e2[:], in_=elog[:],
                        func=mybir.ActivationFunctionType.Exp, scale=cfg.NEG,
                    )
                    # p = exp(lrelu(x)) = max(exp(x), exp(0.2x)); write the max
                    # straight into ph's ones columns
                    ph = php.tile([P, NG, 516], BF, tag="ph")
                    nc.vector.tensor_tensor(
                        out=ph[:, :, 256:258], in0=e1[:, :, 0:2], in1=e2[:, :, 0:2],
                        op=mybir.AluOpType.max,
                    )
                    nc.vector.tensor_tensor(
                        out=ph[:, :, 514:516], in0=e1[:, :, 2:4], in1=e2[:, :, 2:4],
                        op=mybir.AluOpType.max,
                    )
                    nc.vector.tensor_tensor(
                        out=ph[:, :, 0:256].rearrange("p g (h c) -> p g h c", c=C),
                        in0=hg[:, 0:NG, 0:256].rearrange(
                            "p g (h c) -> p g h c", c=C
                        ),
                        in1=ph[:, :, 256:258].to_broadcast([P, NG, 2, C]),
                        op=mybir.AluOpType.mult,
                    )
                    nc.vector.tensor_tensor(
                        out=ph[:, :, 258:514].rearrange("p g (h c) -> p g h c", c=C),
                        in0=hg[:, 0:NG, 258:514].rearrange(
                            "p g (h c) -> p g h c", c=C
                        ),
                        in1=ph[:, :, 514:516].to_broadcast([P, NG, 2, C]),
                        op=mybir.AluOpType.mult,
                    )
                    # aggregate to dsts
                    ps1 = aps.tile([P, 258], F32, tag="ps1")
                    ps2 = aps.tile([P, 258], F32, tag="ps2")
                    for g in range(NG):
                        nc.tensor.matmul(
                            out=ps1[:], lhsT=mp[:, g, 0:P], rhs=ph[:, g, 0:258],
                            start=(g == 0), stop=(g == NG - 1),
                        )
                        nc.tensor.matmul(
                            out=ps2[:], lhsT=mp[:, g, 0:P], rhs=ph[:, g, 258:516],
                            start=(g == 0), stop=(g == NG - 1),
                        )
                    # softmax denominators -> reciprocal
                    rc = ssb.tile([P, H], F32, tag="rc")
                    nc.vector.reciprocal(out=rc[:, 0:2], in_=ps1[:, 256:258])
                    nc.vector.reciprocal(out=rc[:, 2:4], in_=ps2[:, 256:258])
                    v = elup.tile([P, H1], F32, tag="v")
                    nc.vector.tensor_tensor(
                        out=v[:, 0:256].rearrange("p (h c) -> p h c", c=C),
                        in0=ps1[:, 0:256].rearrange("p (h c) -> p h c", c=C),
                        in1=rc[:, 0:2].to_broadcast([P, 2, C]),
                        op=mybir.AluOpType.mult,
                    )
                    nc.vector.tensor_tensor(
                        out=v[:, 256:512].rearrange("p (h c) -> p h c", c=C),
                        in0=ps2[:, 0:256].rearrange("p (h c) -> p h c", c=C),
                        in1=rc[:, 2:4].to_broadcast([P, 2, C]),
                        op=mybir.AluOpType.mult,
                    )
                    if meta["b1_nonzero"]:
                        nc.vector.tensor_tensor(
                            out=v[:], in0=v[:], in1=b1_sb[:],
                            op=mybir.AluOpType.add,
                        )
                    # ELU -> bf16:  elu(v) = relu(v) + exp(-relu(-v)) - 1
                    rneg = elup.tile([P, H1], F32, tag="rneg")
                    nc.scalar.activation(
                        out=rneg[:], in_=v[:],
                        func=mybir.ActivationFunctionType.Relu, scale=-1.0,
                    )
                    sexp = elup.tile([P, H1], F32, tag="sexp")
                    nc.scalar.activation(
                        out=sexp[:], in_=rneg[:],
                        func=mybir.ActivationFunctionType.Exp, scale=-1.0,
                    )
                    rpos = elup.tile([P, H1], F32, tag="rpos")
                    nc.scalar.activation(
                        out=rpos[:], in_=v[:],
                        func=mybir.ActivationFunctionType.Relu,
                    )
                    nc.vector.tensor_tensor(
                        out=sexp[:], in0=rpos[:], in1=sexp[:],
                        op=mybir.AluOpType.add,
                    )
                    h2bf = elup.tile([P, H1], BF, tag="h2bf")
                    nc.scalar.activation(
                        out=h2bf[:], in_=sexp[:],
                        func=mybir.ActivationFunctionType.Copy, bias=-1.0,
                    )
                    # transpose h2, W2ext matmul
                    h2p = tps.tile([P, OC + 2], F32, tag="h2p")
                    for j in range(KH1):
                        tp = tps.tile([P, P], BF, tag="tp")
                        nc.tensor.transpose(
                            out=tp[:], in_=h2bf[:, j * P : (j + 1) * P],
                            identity=ident[:],
                        )
                        h2t = ssb.tile([P, P], BF, tag="h2t")
                        nc.scalar.activation(
                            out=h2t[:], in_=tp[:],
                            func=mybir.ActivationFunctionType.Copy,
                        )
                        nc.tensor.matmul(
                            out=h2p[:], lhsT=h2t[:], rhs=w2eh[:, j, :],
                            start=(j == 0), stop=(j == KH1 - 1),
                        )
                    stage2 = stages2[t % NSTAGE]
                    nc.scalar.activation(
                        out=stage2[:, 0:OC], in_=h2p[:, 0:OC],
                        func=mybir.ActivationFunctionType.Copy,
                    )
                    nc.vector.tensor_copy(
                        out=stage2[:, cfg.R2_ES : cfg.R2_ES + 4].bitcast(F32),
                        in_=h2p[:, OC : OC + 2],
                    )
                    nc.sync.dma_start(
                        out=t2shard[t * P : (t + 1) * P, :], in_=stage2[:]
                    )
                    # chunked allgather as soon as a chunk's tiles are done
                    if "C" in phases and chunked_cc and (t + 1) in CBOUNDS:
                        k = CBOUNDS.index(t + 1) - 1
                        lo, hi = CBOUNDS[k], CBOUNDS[k + 1]
                        nc.gpsimd.collective_compute(
                            "AllGather",
                            mybir.AluOpType.bypass,
                            replica_groups=[list(range(NCORES))],
                            ins=[t2shard[lo * P : hi * P, :].opt()],
                            outs=[
                                t2full[lo * P * NCORES : hi * P * NCORES, :].opt()
                            ],
                        )

              if "C" in phases and not chunked_cc:
                for k in range(len(CBOUNDS) - 1):
                    lo, hi = CBOUNDS[k], CBOUNDS[k + 1]
                    nc.gpsimd.collective_compute(
                        "AllGather",
                        mybir.AluOpType.bypass,
                        replica_groups=[list(range(NCORES))],
                        ins=[t2shard[lo * P : hi * P, :].opt()],
                        outs=[t2full[lo * P * NCORES : hi * P * NCORES, :].opt()],
                    )

              # ---- layer 2 aggregation -> output ----
              with (
                tc.tile_pool(name="hg2_sb", bufs=3) as hgp2,
                tc.tile_pool(name="accD_ps", bufs=2, space="PSUM") as aps,
              ):
                for t in range(T if "D" in phases else 0):
                    hg2 = hgp2.tile([P, NG, ROW2], BF, tag="hg2")
                    for ci, c0 in enumerate(range(0, NG, 8)):
                        gch = min(8, NG - c0)
                        nc.gpsimd.dma_gather(
                            out_ap=hg2[:, c0 : c0 + gch, :],
                            in_ap=t2full[:],
                            idxs_ap=gidx2[:, t, c0 * 8 : (c0 + gch) * 8],
                            num_idxs=gch * P,
                            num_idxs_reg=gch * P,
                            elem_size=ROW2,
                            queue_num=(t * 3 + ci) % NQUEUES,
                        )
                    mp = msb.tile([P, NG, 2 * P], BF, tag="mp")
                    nc.sync.dma_start(
                        out=mp[:],
                        in_=maskw_d[:, t * NG * 2 * P : (t + 1) * NG * 2 * P],
                    )
                    ed2r = ssb.tile([P, 2], BF, tag="ed2r")
                    nc.sync.dma_start(
                        out=ed2r[:],
                        in_=t2shard[t * P : (t + 1) * P, cfg.R2_ED : cfg.R2_ED + 2],
                    )
                    ed2bf = ssb.tile([P, 1], BF, tag="ed2bf")
                    nc.scalar.activation(
                        out=ed2bf[:],
                        in_=ed2r[:].bitcast(F32),
                        func=mybir.ActivationFunctionType.Copy,
                    )
                    ep2 = aps.tile([P, NG], F32, tag="ep2")
                    for g in range(NG):
                        nc.tensor.matmul(
                            out=ep2[:, g : g + 1],
                            lhsT=mp[:, g, P : 2 * P],
                            rhs=ed2bf[:],
                            start=True, stop=True,
                        )
                    elog2 = ssb.tile([P, NG, 1], F32, tag="elog2")
                    nc.vector.tensor_tensor(
                        out=elog2[:],
                        in0=hg2[:, 0:NG, cfg.R2_ES : cfg.R2_ES + 2].bitcast(F32),
                        in1=ep2[:].rearrange("p (g h) -> p g h", h=1),
                        op=mybir.AluOpType.add,
                    )
                    e21 = ssb.tile([P, NG, 1], BF, tag="e21")
                    nc.scalar.activation(
                        out=e21[:], in_=elog2[:],
                        func=mybir.ActivationFunctionType.Exp,
                    )
                    e22 = ssb.tile([P, NG, 1], BF, tag="e22")
                    nc.scalar.activation(
                        out=e22[:], in_=elog2[:],
                        func=mybir.ActivationFunctionType.Exp, scale=cfg.NEG,
                    )
                    p2bf = ssb.tile([P, NG, 1], BF, tag="p2bf")
                    nc.vector.tensor_tensor(
                        out=p2bf[:], in0=e21[:], in1=e22[:],
                        op=mybir.AluOpType.max,
                    )
                    ph2 = ssb.tile([P, NG, OC + 1], BF, tag="ph2")
                    nc.vector.tensor_tensor(
                        out=ph2[:],
                        in0=hg2[:, 0:NG, 0 : OC + 1],
                        in1=p2bf[:, :, 0].to_broadcast([P, NG, OC + 1]),
                        op=mybir.AluOpType.mult,
                    )
                    ps3 = aps.tile([P, OC + 1], F32, tag="ps3")
                    for g in range(NG):
                        nc.tensor.matmul(
                            out=ps3[:], lhsT=mp[:, g, 0:P], rhs=ph2[:, g, :],
                            start=(g == 0), stop=(g == NG - 1),
                        )
                    rc2 = ssb.tile([P, 1], F32, tag="rc2")
                    nc.vector.reciprocal(out=rc2[:], in_=ps3[:, OC : OC + 1])
                    outsb = ssb.tile([P, OC], F32, tag="outsb")
                    nc.vector.tensor_tensor(
                        out=outsb[:],
                        in0=ps3[:, 0:OC],
                        in1=rc2[:].to_broadcast([P, OC]),
                        op=mybir.AluOpType.mult,
                    )
                    if meta["b2_nonzero"]:
                        nc.vector.tensor_tensor(
                            out=outsb[:], in0=outsb[:], in1=b2_sb[:],
                            op=mybir.AluOpType.add,
                        )
                    nc.sync.dma_start(
                        out=out_d[t * P : (t + 1) * P, :], in_=outsb[:]
                    )

    nc.compile()
    return nc


def _default_cfg(n=25000, in_ch=256, hid=128, heads=4, out_ch=64, ng=None,
                 edge_index=None):
    if ng is None:
        N = n
        ei = np.asarray(edge_index, np.int64)
        dst = np.concatenate([ei[1], np.arange(N, dtype=np.int64)])
        counts = np.bincount(dst // P, minlength=-(-n // (P * NCORES)) * NCORES)
        ng = int(np.ceil(counts.max() / P))
    return GATConfig(n, in_ch, hid, heads, out_ch, 0.2, ng)


def run(cfg, inputs, trace=False, tmpdir=None):
    in_maps, meta = host_prep(
        cfg,
        inputs["x"], inputs["edge_index"],
        inputs["W1"], inputs["a_src1"], inputs["a_dst1"], inputs["b1"],
        inputs["W2"], inputs["a_src2"], inputs["a_dst2"], inputs["b2"],
    )
    if meta["b1_nonzero"]:
        for m in in_maps:
            m["b1"] = np.tile(meta["b1"].reshape(1, -1), (P, 1))
    if meta["b2_nonzero"]:
        for m in in_maps:
            m["b2"] = np.tile(meta["b2"].reshape(1, -1), (P, 1))
    nc = build_program(cfg, meta)
    res = run_bass_kernel_spmd(
        nc,
        in_maps,
        core_ids=list(range(NCORES)),
        trace=trace,
        tmpdir=tmpdir,
    )
    shards = [res.results[c]["out"] for c in range(NCORES)]
    full = np.concatenate(shards, axis=0)[: cfg.N]
    return full, res


def kernel(**inputs):
    cfg = _default_cfg(
        n=inputs["x"].shape[0],
        in_ch=inputs["x"].shape[1],
        hid=inputs["a_src1"].shape[1],
        heads=inputs["a_src1"].shape[0],
        out_ch=inputs["a_src2"].shape[1],
        edge_index=inputs["edge_index"],
    )
    out, _ = run(cfg, inputs)
    return out.astype(np.float32)


# revision 4
# speedup vs baseline: 1.0141x; 1.0141x over previous
"""Two-layer GAT (PyG GATConv semantics) on 8 Trainium2 NeuronCores — v2.

Strategy (graph/data parallel): dst nodes range-sharded across 8 cores;
every core redundantly computes the full layer-1 node-feature table
(h = x @ W1ext, with attention projections es/ed folded in as extra
columns); edges sorted by dst and bucketed per 128-dst tile.

v2 batches all per-edge-group elementwise work into per-tile ops
(the v1 kernel spent 1.5 ms in ~5000 tiny DVE instructions), folds the
dst-row fetch into the main gather, loads each tile's one-hot masks in
a single DMA, and pipelines the layer-2 table AllGather in 5-tile
chunks under the layer-1 compute.

Row table layouts (bf16 elements):
  L1 row (640): [h0 h1 (256) | 1 1 (2) | h2 h3 (256) | 1 1 (2) |
                 es f32 x4 (516:524) | ed f32 x4 (524:532) | pad]
  The ones columns ride along in the weighted-feature matmul so a single
  pair of one-hot matmuls yields both the attention-weighted sums and the
  softmax denominators.
  L2 row (128): [h2w2 (64) | 1 (1) | pad (1) | es2 f32 (66:68) |
                 ed2 f32 (68:70) | pad]
  t2full rows are permuted: node (c,t,p) with b=t//5, q=t%5 lives at row
  b*5120 + c*640 + q*128 + p, so each 5-tile AllGather chunk lands
  contiguously.
"""

import sys

for _p in ("/opt/trn_rl_repo",):
    if _p not in sys.path:
        sys.path.insert(0, _p)

import numpy as np
import ml_dtypes

import concourse.bacc as bacc
import concourse.bass as bass
import concourse.mybir as mybir
import concourse.tile as tile
from concourse import library_config
from concourse._compat import axon_active
from concourse.bass_utils import run_bass_kernel_spmd
from concourse.masks import make_identity

BF16 = ml_dtypes.bfloat16
F32 = mybir.dt.float32
BF = mybir.dt.bfloat16
P = 128
NCORES = 8
CBOUNDS = [0, 5, 10, 15, 20, 24, 25]  # AllGather chunk tile boundaries
NQUEUES = 4  # SWDGE queues for gather desc-gen


class GATConfig:
    def __init__(self, n, in_ch, hid, heads, out_ch, neg_slope, ng):
        self.N = n
        self.NPAD = -(-n // (P * NCORES)) * (P * NCORES)
        self.SHARD = self.NPAD // NCORES
        self.T = self.SHARD // P              # dst tiles per core
        self.NT = self.NPAD // P              # node tiles (phase A)
        self.IN_CH = in_ch
        self.KIN = in_ch // P
        self.HID = hid                        # 128
        self.HEADS = heads                    # 4
        self.OUT_CH = out_ch                  # 64
        self.NEG = neg_slope
        self.NG = ng                          # edge groups per dst tile
        self.H1 = heads * hid                 # 512
        # L1 row layout (see module docstring)
        self.R1_ES = 516
        self.R1_ED = 524
        self.ROW1 = 640
        # L2 row layout
        self.R2_ES = out_ch + 2               # 66
        self.R2_ED = out_ch + 4               # 68
        self.ROW2 = 128
        self.KH1 = self.H1 // P               # 4
        assert heads == 4 and hid == 128 and in_ch % P == 0


def _wrap_idx(flat):
    """int16 flat index list -> [128, len/16] wrapped layout for dma_gather."""
    n = len(flat)
    assert n % 16 == 0
    w = np.asarray(flat, np.int16).reshape(n // 16, 16).T  # [16, n/16]
    return np.tile(w, (8, 1))                              # [128, n/16]


def _t2row(node, T):
    """t2full row index for a global node id (chunked-allgather layout)."""
    node = np.asarray(node)
    c = node // (T * P)
    r = node % (T * P)
    t = r // P
    p = r % P
    cb = np.asarray(CBOUNDS)
    k = np.searchsorted(cb, t, side="right") - 1
    w = (cb[k + 1] - cb[k]) * P
    return cb[k] * P * NCORES + c * w + (t - cb[k]) * P + p


def host_prep(cfg, x, edge_index, W1, a_src1, a_dst1, b1, W2, a_src2, a_dst2, b2):
    """Build all per-core input arrays. Returns (in_maps, meta)."""
    N, NPAD = cfg.N, cfg.NPAD
    H, C, OC = cfg.HEADS, cfg.HID, cfg.OUT_CH
    T, NG = cfg.T, cfg.NG

    # --- weights: W1 natural + es/ed projection columns appended --------
    W1 = np.asarray(W1, np.float32)
    W2 = np.asarray(W2, np.float32)
    a_src1 = np.asarray(a_src1, np.float32)
    a_dst1 = np.asarray(a_dst1, np.float32)
    a_src2 = np.asarray(a_src2, np.float32)
    a_dst2 = np.asarray(a_dst2, np.float32)
    w1ext = np.zeros((cfg.IN_CH, cfg.H1 + 8), np.float32)
    w1ext[:, : cfg.H1] = W1
    for h in range(H):
        w1ext[:, cfg.H1 + h] = W1[:, h * C : (h + 1) * C] @ a_src1[h]
        w1ext[:, cfg.H1 + 4 + h] = W1[:, h * C : (h + 1) * C] @ a_dst1[h]
    w1eh = np.ascontiguousarray(
        w1ext.reshape(cfg.KIN, P, cfg.H1 + 8).transpose(1, 0, 2)
    ).astype(BF16)                                        # [128, KIN, 520]

    w2ext = np.zeros((cfg.H1, OC + 2), np.float32)
    w2ext[:, :OC] = W2
    w2ext[:, OC] = W2 @ a_src2[0]
    w2ext[:, OC + 1] = W2 @ a_dst2[0]
    w2eh = np.ascontiguousarray(
        w2ext.reshape(cfg.KH1, P, OC + 2).transpose(1, 0, 2)
    ).astype(BF16)                                        # [128, KH1, 66]

    # --- x, transposed+tiled for lhsT ------------------------------------
    xp = np.zeros((NPAD, cfg.IN_CH), np.float32)
    xp[:N] = np.asarray(x, np.float32)
    xth = np.ascontiguousarray(
        xp.reshape(cfg.NT, P, cfg.KIN, P).transpose(3, 0, 2, 1)
    ).reshape(P, cfg.NT * cfg.KIN * P).astype(BF16)

    # --- edges ------------------------------------------------------------
    ei = np.asarray(edge_index, np.int64)
    loop = np.arange(N, dtype=np.int64)
    src = np.concatenate([ei[0], loop])
    dst = np.concatenate([ei[1], loop])
    order = np.argsort(dst, kind="stable")
    src_s = src[order].astype(np.int32)
    dst_s = dst[order].astype(np.int32)
    gtiles = cfg.NT
    counts = np.bincount(dst_s // P, minlength=gtiles)
    ng = int(np.ceil(counts.max() / P)) if counts.max() else 1
    assert ng <= NG, f"data needs NG={ng} > configured {NG}"
    ET = NG * P
    starts = np.concatenate([[0], np.cumsum(counts)])

    t2map = _t2row(np.arange(NPAD), T)                    # node -> t2full row

    gidx = np.zeros((NCORES, T, P, (NG + 1) * 8), np.int16)
    gidx2 = np.zeros((NCORES, T, P, (NG + 1) * 8), np.int16)
    maskw = np.zeros((NCORES, P, T, NG, 2 * P), BF16)
    eye = np.arange(P, dtype=np.int32)
    for gt in range(gtiles):
        c, t = divmod(gt, T)
        lo, hi = starts[gt], starts[gt + 1]
        k = hi - lo
        idx = np.zeros(ET + P, np.int32)
        idx[:k] = src_s[lo:hi]
        idx[ET : ET + P] = gt * P + eye                   # own dst rows
        dl = np.full(ET, -1, np.int32)
        dl[:k] = dst_s[lo:hi] - gt * P
        # pad dst nodes get a dummy self-edge so their softmax denom is > 0
        pads = eye[gt * P + eye >= N]
        assert k + len(pads) <= ET
        idx[k : k + len(pads)] = gt * P + pads
        dl[k : k + len(pads)] = pads
        gidx[c, t] = _wrap_idx(idx.astype(np.int16))
        gidx2[c, t, :, : NG * 8] = _wrap_idx(t2map[idx[:ET]].astype(np.int16))
        m = dl.reshape(NG, P, 1) == eye.reshape(1, 1, P)  # [NG, e, d]
        maskw[c, :, t, :, :P] = m.transpose(1, 0, 2)      # [e, NG, d]
        maskw[c, :, t, :, P:] = m.transpose(2, 0, 1)      # [d, NG, e]

    in_maps = []
    for c in range(NCORES):
        in_maps.append(
            {
                "xth": xth,
                "w1eh": w1eh,
                "w2eh": w2eh,
                "gidx": gidx[c].reshape(T * P, (NG + 1) * 8),
                "gidx2": gidx2[c].reshape(T * P, (NG + 1) * 8),
                "maskw": maskw[c].reshape(P, T * NG * 2 * P),
            }
        )
    meta = {
        "b1_nonzero": bool(np.any(np.asarray(b1))),
        "b2_nonzero": bool(np.any(np.asarray(b2))),
        "b1": np.asarray(b1, np.float32),
        "b2": np.asarray(b2, np.float32),
    }
    return in_maps, meta


def build_program(cfg, meta, phases="ABCD", chunked_cc=True):
    under_axon = axon_active()
    nc = bacc.Bacc(
        "TRN2",
        target_bir_lowering=False,
        debug=not under_axon,
        num_devices=NCORES,
        dynamic_dma_scratch_size=65536,
        num_swdge_queues=NQUEUES,
    )
    H, C, OC, NG, T = cfg.HEADS, cfg.HID, cfg.OUT_CH, cfg.NG, cfg.T
    H1, KIN, KH1 = cfg.H1, cfg.KIN, cfg.KH1
    ROW1, ROW2 = cfg.ROW1, cfg.ROW2
    ACHUNK = 25                               # phase-A node tiles per x chunk

    xth_d = nc.dram_tensor("xth", [P, cfg.NT * KIN * P], BF, kind="ExternalInput")
    w1eh_d = nc.dram_tensor("w1eh", [P, KIN, H1 + 8], BF, kind="ExternalInput")
    w2eh_d = nc.dram_tensor("w2eh", [P, KH1, OC + 2], BF, kind="ExternalInput")
    gidx_d = nc.dram_tensor("gidx", [T * P, (NG + 1) * 8], mybir.dt.int16,
                            kind="ExternalInput")
    gidx2_d = nc.dram_tensor("gidx2", [T * P, (NG + 1) * 8], mybir.dt.int16,
                             kind="ExternalInput")
    maskw_d = nc.dram_tensor("maskw", [P, T * NG * 2 * P], BF,
                             kind="ExternalInput")
    out_d = nc.dram_tensor("out", [cfg.SHARD, OC], F32, kind="ExternalOutput")

    table1 = nc.dram_tensor("table1", [cfg.NPAD, ROW1], BF)
    t2shard = nc.dram_tensor("t2shard", [cfg.SHARD, ROW2], BF)
    t2full = nc.dram_tensor("t2full", [cfg.NPAD, ROW2], BF, addr_space="Shared")

    if meta["b1_nonzero"]:
        b1_d = nc.dram_tensor("b1", [P, H1], F32, kind="ExternalInput")
    if meta["b2_nonzero"]:
        b2_d = nc.dram_tensor("b2", [P, OC], F32, kind="ExternalInput")

    with tile.TileContext(nc) as tc:
        nc.gpsimd.load_library(library_config.mlp)

        with tc.tile_pool(name="persist", bufs=1) as pp:
            w1eh = pp.tile([P, KIN, H1 + 8], BF)
            nc.sync.dma_start(out=w1eh[:], in_=w1eh_d[:])
            w2eh = pp.tile([P, KH1, OC + 2], BF)
            nc.sync.dma_start(out=w2eh[:], in_=w2eh_d[:])
            gidx = pp.tile([P, T, (NG + 1) * 8], mybir.dt.int16)
            nc.sync.dma_start(
                out=gidx[:], in_=gidx_d[:].rearrange("(t p) s -> p t s", p=P)
            )
            gidx2 = pp.tile([P, T, (NG + 1) * 8], mybir.dt.int16)
            nc.sync.dma_start(
                out=gidx2[:], in_=gidx2_d[:].rearrange("(t p) s -> p t s", p=P)
            )
            ident = pp.tile([P, P], BF)
            make_identity(nc, ident[:])
            if meta["b1_nonzero"]:
                b1_sb = pp.tile([P, H1], F32)
                nc.sync.dma_start(out=b1_sb[:], in_=b1_d[:])
            if meta["b2_nonzero"]:
                b2_sb = pp.tile([P, OC], F32)
                nc.sync.dma_start(out=b2_sb[:], in_=b2_d[:])

            # persistent stage buffers with ones/pad pre-set
            NSTAGE = 3
            stages = []
            for i in range(NSTAGE):
                s = pp.tile([P, 2, ROW1], BF, name=f"stage{i}")
                nc.vector.memset(s[:, :, 256:258], 1.0)
                nc.vector.memset(s[:, :, 514:516], 1.0)
                nc.vector.memset(s[:, :, 532:ROW1], 0.0)
                stages.append(s)
            stages2 = []
            for i in range(NSTAGE):
                s = pp.tile([P, ROW2], BF, name=f"stage2_{i}")
                nc.vector.memset(s[:, OC : OC + 2], 0.0)
                nc.vector.memset(s[:, OC : OC + 1], 1.0)
                nc.vector.memset(s[:, cfg.R2_ED + 2 :], 0.0)
                stages2.append(s)

            # ---------------- Phase A: h table ---------------------------
            if "A" not in phases:
                raise ValueError("phase A required")
            with (
                tc.tile_pool(name="xc_pool", bufs=2) as xcp,
                tc.tile_pool(name="pa_ps", bufs=2, space="PSUM") as pa_ps,
            ):
                for c0 in range(0, cfg.NT, ACHUNK):
                    xc = xcp.tile([P, ACHUNK * KIN * P], BF, tag="xc")
                    nc.sync.dma_start(
                        out=xc[:],
                        in_=xth_d[:, c0 * KIN * P : (c0 + ACHUNK) * KIN * P],
                    )
                    for i in range(ACHUNK):
                        nt = c0 + i
                        ps01 = pa_ps.tile([P, 256], F32, tag="ps01")
                        ps23 = pa_ps.tile([P, 264], F32, tag="ps23")
                        for j in range(KIN):
                            lhs = xc[:, (i * KIN + j) * P : (i * KIN + j + 1) * P]
                            nc.tensor.matmul(
                                out=ps01[:], lhsT=lhs, rhs=w1eh[:, j, 0:256],
                                start=(j == 0), stop=(j == KIN - 1),
                            )
                            nc.tensor.matmul(
                                out=ps23[:], lhsT=lhs, rhs=w1eh[:, j, 256:520],
                                start=(j == 0), stop=(j == KIN - 1),
                            )
                        stage = stages[(nt // 2) % NSTAGE]
                        half = nt % 2
                        nc.scalar.activation(
                            out=stage[:, half, 0:256], in_=ps01[:],
                            func=mybir.ActivationFunctionType.Copy,
                        )
                        nc.vector.tensor_copy(
                            out=stage[:, half, 258:514], in_=ps23[:, 0:256]
                        )
                        nc.scalar.activation(
                            out=stage[:, half, 516:532].bitcast(F32),
                            in_=ps23[:, 256:264],
                            func=mybir.ActivationFunctionType.Copy,
                        )
                        if half == 1:
                            nc.sync.dma_start(
                                out=table1[(nt - 1) * P : (nt + 1) * P, :].rearrange(
                                    "(t p) r -> p t r", p=P
                                ),
                                in_=stage[:],
                            )

            # ---------------- Phases B/C/D --------------------------------
            with (
                tc.tile_pool(name="mask_sb", bufs=2) as msb,
                tc.tile_pool(name="small_sb", bufs=2) as ssb,
            ):
              with (
                tc.tile_pool(name="hg_sb", bufs=2) as hgp,
                tc.tile_pool(name="hg2_sb", bufs=3) as hgp2,
                tc.tile_pool(name="ph_sb", bufs=2) as php,
                tc.tile_pool(name="elu_sb", bufs=1) as elup,
                tc.tile_pool(name="acc_ps", bufs=2, space="PSUM") as aps,
                tc.tile_pool(name="tp_ps", bufs=1, space="PSUM") as tps,
              ):
                # ---- layer 1 aggregation + table2 rows + chunked gather --
                for t in range(T if "B" in phases else 0):
                    hg = hgp.tile([P, NG + 1, ROW1], BF, tag="hg")
                    for ci, c0 in enumerate(range(0, NG + 1, 4)):
                        gch = min(4, NG + 1 - c0)
                        nc.gpsimd.dma_gather(
                            out_ap=hg[:, c0 : c0 + gch, :],
                            in_ap=table1[:],
                            idxs_ap=gidx[:, t, c0 * 8 : (c0 + gch) * 8],
                            num_idxs=gch * P,
                            num_idxs_reg=gch * P,
                            elem_size=ROW1,
                            queue_num=(t * 5 + ci) % NQUEUES,
                        )
                    mp = msb.tile([P, NG, 2 * P], BF, tag="mp")
                    nc.sync.dma_start(
                        out=mp[:],
                        in_=maskw_d[:, t * NG * 2 * P : (t + 1) * NG * 2 * P],
                    )
                    # ed for this tile's dsts (from the appended dst rows)
                    edbf = ssb.tile([P, H], BF, tag="edbf")
                    nc.scalar.activation(
                        out=edbf[:],
                        in_=hg[:, NG, cfg.R1_ED : cfg.R1_ED + 8].bitcast(F32),
                        func=mybir.ActivationFunctionType.Copy,
                    )
                    # ed scattered to edge slots: one psum, NG matmuls
                    ep = aps.tile([P, NG * H], F32, tag="ep")
                    for g in range(NG):
                        nc.tensor.matmul(
                            out=ep[:, g * H : (g + 1) * H],
                            lhsT=mp[:, g, P : 2 * P],
                            rhs=edbf[:],
                            start=True, stop=True,
                        )
                    # batched logits chain
                    elog = ssb.tile([P, NG, H], F32, tag="elog")
                    nc.vector.tensor_tensor(
                        out=elog[:],
                        in0=hg[:, 0:NG, cfg.R1_ES : cfg.R1_ES + 8].bitcast(F32),
                        in1=ep[:].rearrange("p (g h) -> p g h", h=H),
                        op=mybir.AluOpType.add,
                    )
                    e1 = ssb.tile([P, NG, H], BF, tag="e1")
                    nc.scalar.activation(
                        out=e1[:], in_=elog[:],
                        func=mybir.ActivationFunctionType.Exp,
                    )
                    e2 = ssb.tile([P, NG, H], BF, tag="e2")
                    nc.scalar.activation(
                        out=e2[:], in_=elog[:],
                        func=mybir.ActivationFunctionType.Exp, scale=cfg.NEG,
                    )
                    # p = exp(lrelu(x)) = max(exp(x), exp(0.2x)); write the max
                    # straight into ph's ones columns
                    ph = php.tile([P, NG, 516], BF, tag="ph")
                    nc.vector.tensor_tensor(
                        out=ph[:, :, 256:258], in0=e1[:, :, 0:2], in1=e2[:, :, 0:2],
                        op=mybir.AluOpType.max,
                    )
                    nc.vector.tensor_tensor(
                        out=ph[:, :, 514:516], in0=e1[:, :, 2:4], in1=e2[:, :, 2:4],
                        op=mybir.AluOpType.max,
                    )
                    for h0, (pc, hc) in enumerate(
                        ((256, 0), (257, 128), (514, 258), (515, 386))
                    ):
                        nc.vector.tensor_tensor(
                            out=ph[:, :, hc : hc + C],
                            in0=hg[:, 0:NG, hc : hc + C],
                            in1=ph[:, :, pc : pc + 1].to_broadcast([P, NG, C]),
                            op=mybir.AluOpType.mult,
                        )
                    # aggregate to dsts
                    ps1 = aps.tile([P, 258], F32, tag="ps1")
                    ps2 = aps.tile([P, 258], F32, tag="ps2")
                    for g in range(NG):
                        nc.tensor.matmul(
                            out=ps1[:], lhsT=mp[:, g, 0:P], rhs=ph[:, g, 0:258],
                            start=(g == 0), stop=(g == NG - 1),
                        )
                        nc.tensor.matmul(
                            out=ps2[:], lhsT=mp[:, g, 0:P], rhs=ph[:, g, 258:516],
                            start=(g == 0), stop=(g == NG - 1),
                        )
                    # softmax denominators -> reciprocal
                    rc = ssb.tile([P, H], F32, tag="rc")
                    nc.vector.reciprocal(out=rc[:, 0:2], in_=ps1[:, 256:258])
                    nc.vector.reciprocal(out=rc[:, 2:4], in_=ps2[:, 256:258])
                    v = elup.tile([P, H1], F32, tag="v")
                    nc.vector.tensor_tensor(
                        out=v[:, 0:256].rearrange("p (h c) -> p h c", c=C),
                        in0=ps1[:, 0:256].rearrange("p (h c) -> p h c", c=C),
                        in1=rc[:, 0:2].to_broadcast([P, 2, C]),
                        op=mybir.AluOpType.mult,
                    )
                    nc.vector.tensor_tensor(
                        out=v[:, 256:512].rearrange("p (h c) -> p h c", c=C),
                        in0=ps2[:, 0:256].rearrange("p (h c) -> p h c", c=C),
                        in1=rc[:, 2:4].to_broadcast([P, 2, C]),
                        op=mybir.AluOpType.mult,
                    )
                    if meta["b1_nonzero"]:
                        nc.vector.tensor_tensor(
                            out=v[:], in0=v[:], in1=b1_sb[:],
                            op=mybir.AluOpType.add,
                        )
                    # ELU -> bf16:  elu(v) = relu(v) + exp(-relu(-v)) - 1
                    rneg = elup.tile([P, H1], F32, tag="rneg")
                    nc.scalar.activation(
                        out=rneg[:], in_=v[:],
                        func=mybir.ActivationFunctionType.Relu, scale=-1.0,
                    )
                    sexp = elup.tile([P, H1], F32, tag="sexp")
                    nc.scalar.activation(
                        out=sexp[:], in_=rneg[:],
                        func=mybir.ActivationFunctionType.Exp, scale=-1.0,
                    )
                    rpos = elup.tile([P, H1], F32, tag="rpos")
                    nc.scalar.activation(
                        out=rpos[:], in_=v[:],
                        func=mybir.ActivationFunctionType.Relu,
                    )
                    nc.vector.tensor_tensor(
                        out=sexp[:], in0=rpos[:], in1=sexp[:],
                        op=mybir.AluOpType.add,
                    )
                    h2bf = elup.tile([P, H1], BF, tag="h2bf")
                    nc.scalar.activation(
                        out=h2bf[:], in_=sexp[:],
                        func=mybir.ActivationFunctionType.Copy, bias=-1.0,
                    )
                    # transpose h2, W2ext matmul
                    h2p = tps.tile([P, OC + 2], F32, tag="h2p")
                    for j in range(KH1):
                        tp = tps.tile([P, P], BF, tag="tp")
                        nc.tensor.transpose(
                            out=tp[:], in_=h2bf[:, j * P : (j + 1) * P],
                            identity=ident[:],
                        )
                        h2t = ssb.tile([P, P], BF, tag="h2t")
                        nc.scalar.activation(
                            out=h2t[:], in_=tp[:],
                            func=mybir.ActivationFunctionType.Copy,
                        )
                        nc.tensor.matmul(
                            out=h2p[:], lhsT=h2t[:], rhs=w2eh[:, j, :],
                            start=(j == 0), stop=(j == KH1 - 1),
                        )
                    stage2 = stages2[t % NSTAGE]
                    nc.scalar.activation(
                        out=stage2[:, 0:OC], in_=h2p[:, 0:OC],
                        func=mybir.ActivationFunctionType.Copy,
                    )
                    nc.vector.tensor_copy(
                        out=stage2[:, cfg.R2_ES : cfg.R2_ES + 4].bitcast(F32),
                        in_=h2p[:, OC : OC + 2],
                    )
                    nc.sync.dma_start(
                        out=t2shard[t * P : (t + 1) * P, :], in_=stage2[:]
                    )
                    # chunked allgather as soon as a chunk's tiles are done
                    if "C" in phases and chunked_cc and (t + 1) in CBOUNDS:
                        k = CBOUNDS.index(t + 1) - 1
                        lo, hi = CBOUNDS[k], CBOUNDS[k + 1]
                        nc.gpsimd.collective_compute(
                            "AllGather",
                            mybir.AluOpType.bypass,
                            replica_groups=[list(range(NCORES))],
                            ins=[t2shard[lo * P : hi * P, :].opt()],
                            outs=[
                                t2full[lo * P * NCORES : hi * P * NCORES, :].opt()
                            ],
                        )

              if "C" in phases and not chunked_cc:
                for k in range(len(CBOUNDS) - 1):
                    lo, hi = CBOUNDS[k], CBOUNDS[k + 1]
                    nc.gpsimd.collective_compute(
                        "AllGather",
                        mybir.AluOpType.bypass,
                        replica_groups=[list(range(NCORES))],
                        ins=[t2shard[lo * P : hi * P, :].opt()],
                        outs=[t2full[lo * P * NCORES : hi * P * NCORES, :].opt()],
                    )

              # ---- layer 2 aggregation -> output ----
              with (
                tc.tile_pool(name="hg2_sb", bufs=3) as hgp2,
                tc.tile_pool(name="accD_ps", bufs=2, space="PSUM") as aps,
              ):
                for t in range(T if "D" in phases else 0):
                    hg2 = hgp2.tile([P, NG, ROW2], BF, tag="hg2")
                    for ci, c0 in enumerate(range(0, NG, 4)):
                        gch = min(4, NG - c0)
                        nc.gpsimd.dma_gather(
                            out_ap=hg2[:, c0 : c0 + gch, :],
                            in_ap=t2full[:],
                            idxs_ap=gidx2[:, t, c0 * 8 : (c0 + gch) * 8],
                            num_idxs=gch * P,
                            num_idxs_reg=gch * P,
                            elem_size=ROW2,
                            queue_num=(t * 5 + ci) % NQUEUES,
                        )
                    mp = msb.tile([P, NG, 2 * P], BF, tag="mp")
                    nc.sync.dma_start(
                        out=mp[:],
                        in_=maskw_d[:, t * NG * 2 * P : (t + 1) * NG * 2 * P],
                    )
                    ed2r = ssb.tile([P, 2], BF, tag="ed2r")
                    nc.sync.dma_start(
                        out=ed2r[:],
                        in_=t2shard[t * P : (t + 1) * P, cfg.R2_ED : cfg.R2_ED + 2],
                    )
                    ed2bf = ssb.tile([P, 1], BF, tag="ed2bf")
                    nc.scalar.activation(
                        out=ed2bf[:],
                        in_=ed2r[:].bitcast(F32),
                        func=mybir.ActivationFunctionType.Copy,
                    )
                    ep2 = aps.tile([P, NG], F32, tag="ep2")
                    for g in range(NG):
                        nc.tensor.matmul(
                            out=ep2[:, g : g + 1],
                            lhsT=mp[:, g, P : 2 * P],
                            rhs=ed2bf[:],
                            start=True, stop=True,
                        )
                    elog2 = ssb.tile([P, NG, 1], F32, tag="elog2")
                    nc.vector.tensor_tensor(
                        out=elog2[:],
                        in0=hg2[:, 0:NG, cfg.R2_ES : cfg.R2_ES + 2].bitcast(F32),
                        in1=ep2[:].rearrange("p (g h) -> p g h", h=1),
                        op=mybir.AluOpType.add,
                    )
                    e21 = ssb.tile([P, NG, 1], BF, tag="e21")
                    nc.scalar.activation(
                        out=e21[:], in_=elog2[:],
                        func=mybir.ActivationFunctionType.Exp,
                    )
                    e22 = ssb.tile([P, NG, 1], BF, tag="e22")
                    nc.scalar.activation(
                        out=e22[:], in_=elog2[:],
                        func=mybir.ActivationFunctionType.Exp, scale=cfg.NEG,
                    )
                    p2bf = ssb.tile([P, NG, 1], BF, tag="p2bf")
                    nc.vector.tensor_tensor(
                        out=p2bf[:], in0=e21[:], in1=e22[:],
                        op=mybir.AluOpType.max,
                    )
                    ph2 = ssb.tile([P, NG, OC + 1], BF, tag="ph2")
                    nc.vector.tensor_tensor(
                        out=ph2[:],
                        in0=hg2[:, 0:NG, 0 : OC + 1],
                        in1=p2bf[:, :, 0].to_broadcast([P, NG, OC + 1]),
                        op=mybir.AluOpType.mult,
                    )
                    ps3 = aps.tile([P, OC + 1], F32, tag="ps3")
                    for g in range(NG):
                        nc.tensor.matmul(
                            out=ps3[:], lhsT=mp[:, g, 0:P], rhs=ph2[:, g, :],
                            start=(g == 0), stop=(g == NG - 1),
                        )
                    rc2 = ssb.tile([P, 1], F32, tag="rc2")
                    nc.vector.reciprocal(out=rc2[:], in_=ps3[:, OC : OC + 1])
                    outsb = ssb.tile([P, OC], F32, tag="outsb")
                    nc.vector.tensor_tensor(
                        out=outsb[:],
                        in0=ps3[:, 0:OC],
                        in1=rc2[:].to_broadcast([P, OC]),
                        op=mybir.AluOpType.mult,
                    )
                    if meta["b2_nonzero"]:
                        nc.vector.tensor_tensor(
                            out=outsb[:], in0=outsb[:], in1=b2_sb[:],
                            op=mybir.AluOpType.add,
                        )
                    nc.sync.dma_start(
                        out=out_d[t * P : (t + 1) * P, :], in_=outsb[:]
                    )

    nc.compile()
    return nc


def _default_cfg(n=25000, in_ch=256, hid=128, heads=4, out_ch=64, ng=None,
                 edge_index=None):
    if ng is None:
        N = n
        ei = np.asarray(edge_index, np.int64)
        dst = np.concatenate([ei[1], np.arange(N, dtype=np.int64)])
        counts = np.bincount(dst // P, minlength=-(-n // (P * NCORES)) * NCORES)
        ng = int(np.ceil(counts.max() / P))
    return GATConfig(n, in_ch, hid, heads, out_ch, 0.2, ng)


def run(cfg, inputs, trace=False, tmpdir=None):
    in_maps, meta = host_prep(
        cfg,
        inputs["x"], inputs["edge_index"],
        inputs["W1"], inputs["a_src1"], inputs["a_dst1"], inputs["b1"],
        inputs["W2"], inputs["a_src2"], inputs["a_dst2"], inputs["b2"],
    )
    if meta["b1_nonzero"]:
        for m in in_maps:
            m["b1"] = np.tile(meta["b1"].reshape(1, -1), (P, 1))
    if meta["b2_nonzero"]:
        for m in in_maps:
            m["b2"] = np.tile(meta["b2"].reshape(1, -1), (P, 1))
    nc = build_program(cfg, meta)
    res = run_bass_kernel_spmd(
        nc,
        in_maps,
        core_ids=list(range(NCORES)),
        trace=trace,
        tmpdir=tmpdir,
    )
    shards = [res.results[c]["out"] for c in range(NCORES)]
    full = np.concatenate(shards, axis=0)[: cfg.N]
    return full, res


def kernel(**inputs):
    cfg = _default_cfg(
        n=inputs["x"].shape[0],
        in_ch=inputs["x"].shape[1],
        hid=inputs["a_src1"].shape[1],
        heads=inputs["a_src1"].shape[0],
        out_ch=inputs["a_src2"].shape[1],
        edge_index=inputs["edge_index"],
    )
    out, _ = run(cfg, inputs)
    return out.astype(np.float32)


# revision 5
# speedup vs baseline: 1.0380x; 1.0236x over previous
"""Two-layer GAT (PyG GATConv semantics) on 8 Trainium2 NeuronCores — v2.

Strategy (graph/data parallel): dst nodes range-sharded across 8 cores;
every core redundantly computes the full layer-1 node-feature table
(h = x @ W1ext, with attention projections es/ed folded in as extra
columns); edges sorted by dst and bucketed per 128-dst tile.

v2 batches all per-edge-group elementwise work into per-tile ops
(the v1 kernel spent 1.5 ms in ~5000 tiny DVE instructions), folds the
dst-row fetch into the main gather, loads each tile's one-hot masks in
a single DMA, and pipelines the layer-2 table AllGather in 5-tile
chunks under the layer-1 compute.

Row table layouts (bf16 elements):
  L1 row (640): [h0 h1 (256) | 1 1 (2) | h2 h3 (256) | 1 1 (2) |
                 es f32 x4 (516:524) | ed f32 x4 (524:532) | pad]
  The ones columns ride along in the weighted-feature matmul so a single
  pair of one-hot matmuls yields both the attention-weighted sums and the
  softmax denominators.
  L2 row (128): [h2w2 (64) | 1 (1) | pad (1) | es2 f32 (66:68) |
                 ed2 f32 (68:70) | pad]
  t2full rows are permuted: node (c,t,p) with b=t//5, q=t%5 lives at row
  b*5120 + c*640 + q*128 + p, so each 5-tile AllGather chunk lands
  contiguously.
"""

import sys

for _p in ("/opt/trn_rl_repo",):
    if _p not in sys.path:
        sys.path.insert(0, _p)

import numpy as np
import ml_dtypes

import concourse.bacc as bacc
import concourse.bass as bass
import concourse.mybir as mybir
import concourse.tile as tile
from concourse import library_config
from concourse._compat import axon_active
from concourse.bass_utils import run_bass_kernel_spmd
from concourse.masks import make_identity

BF16 = ml_dtypes.bfloat16
F32 = mybir.dt.float32
BF = mybir.dt.bfloat16
P = 128
NCORES = 8
CBOUNDS = [0, 5, 10, 15, 20, 24, 25]  # AllGather chunk tile boundaries
NQUEUES = 4  # SWDGE queues for gather desc-gen


class GATConfig:
    def __init__(self, n, in_ch, hid, heads, out_ch, neg_slope, ng):
        self.N = n
        self.NPAD = -(-n // (P * NCORES)) * (P * NCORES)
        self.SHARD = self.NPAD // NCORES
        self.T = self.SHARD // P              # dst tiles per core
        self.NT = self.NPAD // P              # node tiles (phase A)
        self.IN_CH = in_ch
        self.KIN = in_ch // P
        self.HID = hid                        # 128
        self.HEADS = heads                    # 4
        self.OUT_CH = out_ch                  # 64
        self.NEG = neg_slope
        self.NG = ng                          # edge groups per dst tile
        self.H1 = heads * hid                 # 512
        # L1 row layout (see module docstring)
        self.R1_ES = 516
        self.R1_ED = 524
        self.ROW1 = 640
        # L2 row layout
        self.R2_ES = out_ch + 2               # 66
        self.R2_ED = out_ch + 4               # 68
        self.ROW2 = 128
        self.KH1 = self.H1 // P               # 4
        assert heads == 4 and hid == 128 and in_ch % P == 0


def _wrap_idx(flat):
    """int16 flat index list -> [128, len/16] wrapped layout for dma_gather."""
    n = len(flat)
    assert n % 16 == 0
    w = np.asarray(flat, np.int16).reshape(n // 16, 16).T  # [16, n/16]
    return np.tile(w, (8, 1))                              # [128, n/16]


def _t2row(node, T):
    """t2full row index for a global node id (chunked-allgather layout)."""
    node = np.asarray(node)
    c = node // (T * P)
    r = node % (T * P)
    t = r // P
    p = r % P
    cb = np.asarray(CBOUNDS)
    k = np.searchsorted(cb, t, side="right") - 1
    w = (cb[k + 1] - cb[k]) * P
    return cb[k] * P * NCORES + c * w + (t - cb[k]) * P + p


def host_prep(cfg, x, edge_index, W1, a_src1, a_dst1, b1, W2, a_src2, a_dst2, b2):
    """Build all per-core input arrays. Returns (in_maps, meta)."""
    N, NPAD = cfg.N, cfg.NPAD
    H, C, OC = cfg.HEADS, cfg.HID, cfg.OUT_CH
    T, NG = cfg.T, cfg.NG

    # --- weights: W1 natural + es/ed projection columns appended --------
    W1 = np.asarray(W1, np.float32)
    W2 = np.asarray(W2, np.float32)
    a_src1 = np.asarray(a_src1, np.float32)
    a_dst1 = np.asarray(a_dst1, np.float32)
    a_src2 = np.asarray(a_src2, np.float32)
    a_dst2 = np.asarray(a_dst2, np.float32)
    w1ext = np.zeros((cfg.IN_CH, cfg.H1 + 8), np.float32)
    w1ext[:, : cfg.H1] = W1
    for h in range(H):
        w1ext[:, cfg.H1 + h] = W1[:, h * C : (h + 1) * C] @ a_src1[h]
        w1ext[:, cfg.H1 + 4 + h] = W1[:, h * C : (h + 1) * C] @ a_dst1[h]
    w1eh = np.ascontiguousarray(
        w1ext.reshape(cfg.KIN, P, cfg.H1 + 8).transpose(1, 0, 2)
    ).astype(BF16)                                        # [128, KIN, 520]

    w2ext = np.zeros((cfg.H1, OC + 2), np.float32)
    w2ext[:, :OC] = W2
    w2ext[:, OC] = W2 @ a_src2[0]
    w2ext[:, OC + 1] = W2 @ a_dst2[0]
    w2eh = np.ascontiguousarray(
        w2ext.reshape(cfg.KH1, P, OC + 2).transpose(1, 0, 2)
    ).astype(BF16)                                        # [128, KH1, 66]

    # --- x, transposed+tiled for lhsT ------------------------------------
    xp = np.zeros((NPAD, cfg.IN_CH), np.float32)
    xp[:N] = np.asarray(x, np.float32)
    xth = np.ascontiguousarray(
        xp.reshape(cfg.NT, P, cfg.KIN, P).transpose(3, 0, 2, 1)
    ).reshape(P, cfg.NT * cfg.KIN * P).astype(BF16)

    # --- edges ------------------------------------------------------------
    ei = np.asarray(edge_index, np.int64)
    loop = np.arange(N, dtype=np.int64)
    src = np.concatenate([ei[0], loop])
    dst = np.concatenate([ei[1], loop])
    order = np.argsort(dst, kind="stable")
    src_s = src[order].astype(np.int32)
    dst_s = dst[order].astype(np.int32)
    gtiles = cfg.NT
    counts = np.bincount(dst_s // P, minlength=gtiles)
    ng = int(np.ceil(counts.max() / P)) if counts.max() else 1
    assert ng <= NG, f"data needs NG={ng} > configured {NG}"
    ET = NG * P
    starts = np.concatenate([[0], np.cumsum(counts)])

    t2map = _t2row(np.arange(NPAD), T)                    # node -> t2full row

    gidx = np.zeros((NCORES, T, P, (NG + 1) * 8), np.int16)
    gidx2 = np.zeros((NCORES, T, P, (NG + 1) * 8), np.int16)
    maskw = np.zeros((NCORES, P, T, NG, 2 * P), BF16)
    eye = np.arange(P, dtype=np.int32)
    for gt in range(gtiles):
        c, t = divmod(gt, T)
        lo, hi = starts[gt], starts[gt + 1]
        k = hi - lo
        idx = np.zeros(ET + P, np.int32)
        idx[:k] = src_s[lo:hi]
        idx[ET : ET + P] = gt * P + eye                   # own dst rows
        dl = np.full(ET, -1, np.int32)
        dl[:k] = dst_s[lo:hi] - gt * P
        # pad dst nodes get a dummy self-edge so their softmax denom is > 0
        pads = eye[gt * P + eye >= N]
        assert k + len(pads) <= ET
        idx[k : k + len(pads)] = gt * P + pads
        dl[k : k + len(pads)] = pads
        gidx[c, t] = _wrap_idx(idx.astype(np.int16))
        gidx2[c, t, :, : NG * 8] = _wrap_idx(t2map[idx[:ET]].astype(np.int16))
        m = dl.reshape(NG, P, 1) == eye.reshape(1, 1, P)  # [NG, e, d]
        maskw[c, :, t, :, :P] = m.transpose(1, 0, 2)      # [e, NG, d]
        maskw[c, :, t, :, P:] = m.transpose(2, 0, 1)      # [d, NG, e]

    in_maps = []
    for c in range(NCORES):
        in_maps.append(
            {
                "xth": xth,
                "w1eh": w1eh,
                "w2eh": w2eh,
                "gidx": gidx[c].reshape(T * P, (NG + 1) * 8),
                "gidx2": gidx2[c].reshape(T * P, (NG + 1) * 8),
                "maskw": maskw[c].reshape(P, T * NG * 2 * P),
            }
        )
    meta = {
        "b1_nonzero": bool(np.any(np.asarray(b1))),
        "b2_nonzero": bool(np.any(np.asarray(b2))),
        "b1": np.asarray(b1, np.float32),
        "b2": np.asarray(b2, np.float32),
    }
    return in_maps, meta


def build_program(cfg, meta, phases="ABCD", chunked_cc=True):
    under_axon = axon_active()
    nc = bacc.Bacc(
        "TRN2",
        target_bir_lowering=False,
        debug=not under_axon,
        num_devices=NCORES,
        dynamic_dma_scratch_size=65536,
        num_swdge_queues=NQUEUES,
    )
    H, C, OC, NG, T = cfg.HEADS, cfg.HID, cfg.OUT_CH, cfg.NG, cfg.T
    H1, KIN, KH1 = cfg.H1, cfg.KIN, cfg.KH1
    ROW1, ROW2 = cfg.ROW1, cfg.ROW2
    ACHUNK = 25                               # phase-A node tiles per x chunk

    xth_d = nc.dram_tensor("xth", [P, cfg.NT * KIN * P], BF, kind="ExternalInput")
    w1eh_d = nc.dram_tensor("w1eh", [P, KIN, H1 + 8], BF, kind="ExternalInput")
    w2eh_d = nc.dram_tensor("w2eh", [P, KH1, OC + 2], BF, kind="ExternalInput")
    gidx_d = nc.dram_tensor("gidx", [T * P, (NG + 1) * 8], mybir.dt.int16,
                            kind="ExternalInput")
    gidx2_d = nc.dram_tensor("gidx2", [T * P, (NG + 1) * 8], mybir.dt.int16,
                             kind="ExternalInput")
    maskw_d = nc.dram_tensor("maskw", [P, T * NG * 2 * P], BF,
                             kind="ExternalInput")
    out_d = nc.dram_tensor("out", [cfg.SHARD, OC], F32, kind="ExternalOutput")

    table1 = nc.dram_tensor("table1", [cfg.NPAD, ROW1], BF)
    t2shard = nc.dram_tensor("t2shard", [cfg.SHARD, ROW2], BF)
    t2full = nc.dram_tensor("t2full", [cfg.NPAD, ROW2], BF, addr_space="Shared")

    if meta["b1_nonzero"]:
        b1_d = nc.dram_tensor("b1", [P, H1], F32, kind="ExternalInput")
    if meta["b2_nonzero"]:
        b2_d = nc.dram_tensor("b2", [P, OC], F32, kind="ExternalInput")

    with tile.TileContext(nc) as tc:
        nc.gpsimd.load_library(library_config.mlp)

        with tc.tile_pool(name="persist", bufs=1) as pp:
            w1eh = pp.tile([P, KIN, H1 + 8], BF)
            nc.sync.dma_start(out=w1eh[:], in_=w1eh_d[:])
            w2eh = pp.tile([P, KH1, OC + 2], BF)
            nc.sync.dma_start(out=w2eh[:], in_=w2eh_d[:])
            gidx = pp.tile([P, T, (NG + 1) * 8], mybir.dt.int16)
            nc.sync.dma_start(
                out=gidx[:], in_=gidx_d[:].rearrange("(t p) s -> p t s", p=P)
            )
            gidx2 = pp.tile([P, T, (NG + 1) * 8], mybir.dt.int16)
            nc.sync.dma_start(
                out=gidx2[:], in_=gidx2_d[:].rearrange("(t p) s -> p t s", p=P)
            )
            ident = pp.tile([P, P], BF)
            make_identity(nc, ident[:])
            if meta["b1_nonzero"]:
                b1_sb = pp.tile([P, H1], F32)
                nc.sync.dma_start(out=b1_sb[:], in_=b1_d[:])
            if meta["b2_nonzero"]:
                b2_sb = pp.tile([P, OC], F32)
                nc.sync.dma_start(out=b2_sb[:], in_=b2_d[:])

            # persistent stage buffers with ones/pad pre-set
            NSTAGE = 3
            stages = []
            for i in range(NSTAGE):
                s = pp.tile([P, 2, ROW1], BF, name=f"stage{i}")
                nc.vector.memset(s[:, :, 256:258], 1.0)
                nc.vector.memset(s[:, :, 514:516], 1.0)
                nc.vector.memset(s[:, :, 532:ROW1], 0.0)
                stages.append(s)
            stages2 = []
            for i in range(NSTAGE):
                s = pp.tile([P, ROW2], BF, name=f"stage2_{i}")
                nc.vector.memset(s[:, OC : OC + 2], 0.0)
                nc.vector.memset(s[:, OC : OC + 1], 1.0)
                nc.vector.memset(s[:, cfg.R2_ED + 2 :], 0.0)
                stages2.append(s)

            # ---------------- Phase A: h table ---------------------------
            if "A" not in phases:
                raise ValueError("phase A required")
            with (
                tc.tile_pool(name="xc_pool", bufs=2) as xcp,
                tc.tile_pool(name="pa_ps", bufs=2, space="PSUM") as pa_ps,
            ):
                for c0 in range(0, cfg.NT, ACHUNK):
                    xc = xcp.tile([P, ACHUNK * KIN * P], BF, tag="xc")
                    nc.sync.dma_start(
                        out=xc[:],
                        in_=xth_d[:, c0 * KIN * P : (c0 + ACHUNK) * KIN * P],
                    )
                    for i in range(ACHUNK):
                        nt = c0 + i
                        ps01 = pa_ps.tile([P, 256], F32, tag="ps01")
                        ps23 = pa_ps.tile([P, 264], F32, tag="ps23")
                        for j in range(KIN):
                            lhs = xc[:, (i * KIN + j) * P : (i * KIN + j + 1) * P]
                            nc.tensor.matmul(
                                out=ps01[:], lhsT=lhs, rhs=w1eh[:, j, 0:256],
                                start=(j == 0), stop=(j == KIN - 1),
                            )
                            nc.tensor.matmul(
                                out=ps23[:], lhsT=lhs, rhs=w1eh[:, j, 256:520],
                                start=(j == 0), stop=(j == KIN - 1),
                            )
                        stage = stages[(nt // 2) % NSTAGE]
                        half = nt % 2
                        nc.scalar.activation(
                            out=stage[:, half, 0:256], in_=ps01[:],
                            func=mybir.ActivationFunctionType.Copy,
                        )
                        nc.vector.tensor_copy(
                            out=stage[:, half, 258:514], in_=ps23[:, 0:256]
                        )
                        nc.scalar.activation(
                            out=stage[:, half, 516:532].bitcast(F32),
                            in_=ps23[:, 256:264],
                            func=mybir.ActivationFunctionType.Copy,
                        )
                        if half == 1:
                            nc.sync.dma_start(
                                out=table1[(nt - 1) * P : (nt + 1) * P, :].rearrange(
                                    "(t p) r -> p t r", p=P
                                ),
                                in_=stage[:],
                            )

            # ---------------- Phases B/C/D --------------------------------
            with (
                tc.tile_pool(name="mask_sb", bufs=2) as msb,
                tc.tile_pool(name="small_sb", bufs=2) as ssb,
            ):
              with (
                tc.tile_pool(name="hg_sb", bufs=2) as hgp,
                tc.tile_pool(name="hg2_sb", bufs=4) as hgp2,
                tc.tile_pool(name="ph_sb", bufs=2) as php,
                tc.tile_pool(name="elu_sb", bufs=1) as elup,
                tc.tile_pool(name="acc_ps", bufs=2, space="PSUM") as aps,
                tc.tile_pool(name="tp_ps", bufs=1, space="PSUM") as tps,
              ):
                # ---- layer 1 aggregation + table2 rows + chunked gather --
                for t in range(T if "B" in phases else 0):
                    hg = hgp.tile([P, NG + 1, ROW1], BF, tag="hg")
                    for ci, c0 in enumerate(range(0, NG + 1, 4)):
                        gch = min(4, NG + 1 - c0)
                        nc.gpsimd.dma_gather(
                            out_ap=hg[:, c0 : c0 + gch, :],
                            in_ap=table1[:],
                            idxs_ap=gidx[:, t, c0 * 8 : (c0 + gch) * 8],
                            num_idxs=gch * P,
                            num_idxs_reg=gch * P,
                            elem_size=ROW1,
                            queue_num=(t * 5 + ci) % NQUEUES,
                        )
                    mp = msb.tile([P, NG, 2 * P], BF, tag="mp")
                    nc.sync.dma_start(
                        out=mp[:],
                        in_=maskw_d[:, t * NG * 2 * P : (t + 1) * NG * 2 * P],
                    )
                    # ed for this tile's dsts (from the appended dst rows)
                    edbf = ssb.tile([P, H], BF, tag="edbf")
                    nc.scalar.activation(
                        out=edbf[:],
                        in_=hg[:, NG, cfg.R1_ED : cfg.R1_ED + 8].bitcast(F32),
                        func=mybir.ActivationFunctionType.Copy,
                    )
                    # ed scattered to edge slots: one psum, NG matmuls
                    ep = aps.tile([P, NG * H], F32, tag="ep")
                    for g in range(NG):
                        nc.tensor.matmul(
                            out=ep[:, g * H : (g + 1) * H],
                            lhsT=mp[:, g, P : 2 * P],
                            rhs=edbf[:],
                            start=True, stop=True,
                        )
                    # batched logits chain
                    elog = ssb.tile([P, NG, H], F32, tag="elog")
                    nc.vector.tensor_tensor(
                        out=elog[:],
                        in0=hg[:, 0:NG, cfg.R1_ES : cfg.R1_ES + 8].bitcast(F32),
                        in1=ep[:].rearrange("p (g h) -> p g h", h=H),
                        op=mybir.AluOpType.add,
                    )
                    e1 = ssb.tile([P, NG, H], BF, tag="e1")
                    nc.scalar.activation(
                        out=e1[:], in_=elog[:],
                        func=mybir.ActivationFunctionType.Exp,
                    )
                    e2 = ssb.tile([P, NG, H], BF, tag="e2")
                    nc.scalar.activation(
                        out=e2[:], in_=elog[:],
                        func=mybir.ActivationFunctionType.Exp, scale=cfg.NEG,
                    )
                    # p = exp(lrelu(x)) = max(exp(x), exp(0.2x)); write the max
                    # straight into ph's ones columns
                    ph = php.tile([P, NG, 516], BF, tag="ph")
                    nc.vector.tensor_tensor(
                        out=ph[:, :, 256:258], in0=e1[:, :, 0:2], in1=e2[:, :, 0:2],
                        op=mybir.AluOpType.max,
                    )
                    nc.vector.tensor_tensor(
                        out=ph[:, :, 514:516], in0=e1[:, :, 2:4], in1=e2[:, :, 2:4],
                        op=mybir.AluOpType.max,
                    )
                    for h0, (pc, hc) in enumerate(
                        ((256, 0), (257, 128), (514, 258), (515, 386))
                    ):
                        nc.vector.tensor_tensor(
                            out=ph[:, :, hc : hc + C],
                            in0=hg[:, 0:NG, hc : hc + C],
                            in1=ph[:, :, pc : pc + 1].to_broadcast([P, NG, C]),
                            op=mybir.AluOpType.mult,
                        )
                    # aggregate to dsts
                    ps1 = aps.tile([P, 258], F32, tag="ps1")
                    ps2 = aps.tile([P, 258], F32, tag="ps2")
                    for g in range(NG):
                        nc.tensor.matmul(
                            out=ps1[:], lhsT=mp[:, g, 0:P], rhs=ph[:, g, 0:258],
                            start=(g == 0), stop=(g == NG - 1),
                        )
                        nc.tensor.matmul(
                            out=ps2[:], lhsT=mp[:, g, 0:P], rhs=ph[:, g, 258:516],
                            start=(g == 0), stop=(g == NG - 1),
                        )
                    # softmax denominators -> reciprocal
                    rc = ssb.tile([P, H], F32, tag="rc")
                    nc.vector.reciprocal(out=rc[:, 0:2], in_=ps1[:, 256:258])
                    nc.vector.reciprocal(out=rc[:, 2:4], in_=ps2[:, 256:258])
                    v = elup.tile([P, H1], F32, tag="v")
                    nc.vector.tensor_tensor(
                        out=v[:, 0:256].rearrange("p (h c) -> p h c", c=C),
                        in0=ps1[:, 0:256].rearrange("p (h c) -> p h c", c=C),
                        in1=rc[:, 0:2].to_broadcast([P, 2, C]),
                        op=mybir.AluOpType.mult,
                    )
                    nc.vector.tensor_tensor(
                        out=v[:, 256:512].rearrange("p (h c) -> p h c", c=C),
                        in0=ps2[:, 0:256].rearrange("p (h c) -> p h c", c=C),
                        in1=rc[:, 2:4].to_broadcast([P, 2, C]),
                        op=mybir.AluOpType.mult,
                    )
                    if meta["b1_nonzero"]:
                        nc.vector.tensor_tensor(
                            out=v[:], in0=v[:], in1=b1_sb[:],
                            op=mybir.AluOpType.add,
                        )
                    # ELU -> bf16:  elu(v) = relu(v) + exp(-relu(-v)) - 1
                    rneg = elup.tile([P, H1], F32, tag="rneg")
                    nc.scalar.activation(
                        out=rneg[:], in_=v[:],
                        func=mybir.ActivationFunctionType.Relu, scale=-1.0,
                    )
                    sexp = elup.tile([P, H1], F32, tag="sexp")
                    nc.scalar.activation(
                        out=sexp[:], in_=rneg[:],
                        func=mybir.ActivationFunctionType.Exp, scale=-1.0,
                    )
                    rpos = elup.tile([P, H1], F32, tag="rpos")
                    nc.scalar.activation(
                        out=rpos[:], in_=v[:],
                        func=mybir.ActivationFunctionType.Relu,
                    )
                    nc.vector.tensor_tensor(
                        out=sexp[:], in0=rpos[:], in1=sexp[:],
                        op=mybir.AluOpType.add,
                    )
                    h2bf = elup.tile([P, H1], BF, tag="h2bf")
                    nc.scalar.activation(
                        out=h2bf[:], in_=sexp[:],
                        func=mybir.ActivationFunctionType.Copy, bias=-1.0,
                    )
                    # transpose h2, W2ext matmul
                    h2p = tps.tile([P, OC + 2], F32, tag="h2p")
                    for j in range(KH1):
                        tp = tps.tile([P, P], BF, tag="tp")
                        nc.tensor.transpose(
                            out=tp[:], in_=h2bf[:, j * P : (j + 1) * P],
                            identity=ident[:],
                        )
                        h2t = ssb.tile([P, P], BF, tag="h2t")
                        nc.scalar.activation(
                            out=h2t[:], in_=tp[:],
                            func=mybir.ActivationFunctionType.Copy,
                        )
                        nc.tensor.matmul(
                            out=h2p[:], lhsT=h2t[:], rhs=w2eh[:, j, :],
                            start=(j == 0), stop=(j == KH1 - 1),
                        )
                    stage2 = stages2[t % NSTAGE]
                    nc.scalar.activation(
                        out=stage2[:, 0:OC], in_=h2p[:, 0:OC],
                        func=mybir.ActivationFunctionType.Copy,
                    )
                    nc.vector.tensor_copy(
                        out=stage2[:, cfg.R2_ES : cfg.R2_ES + 4].bitcast(F32),
                        in_=h2p[:, OC : OC + 2],
                    )
                    nc.sync.dma_start(
                        out=t2shard[t * P : (t + 1) * P, :], in_=stage2[:]
                    )
                    # chunked allgather as soon as a chunk's tiles are done
                    if "C" in phases and chunked_cc and (t + 1) in CBOUNDS:
                        k = CBOUNDS.index(t + 1) - 1
                        lo, hi = CBOUNDS[k], CBOUNDS[k + 1]
                        nc.gpsimd.collective_compute(
                            "AllGather",
                            mybir.AluOpType.bypass,
                            replica_groups=[list(range(NCORES))],
                            ins=[t2shard[lo * P : hi * P, :].opt()],
                            outs=[
                                t2full[lo * P * NCORES : hi * P * NCORES, :].opt()
                            ],
                        )

              if "C" in phases and not chunked_cc:
                for k in range(len(CBOUNDS) - 1):
                    lo, hi = CBOUNDS[k], CBOUNDS[k + 1]
                    nc.gpsimd.collective_compute(
                        "AllGather",
                        mybir.AluOpType.bypass,
                        replica_groups=[list(range(NCORES))],
                        ins=[t2shard[lo * P : hi * P, :].opt()],
                        outs=[t2full[lo * P * NCORES : hi * P * NCORES, :].opt()],
                    )

              # ---- layer 2 aggregation -> output ----
              with (
                tc.tile_pool(name="hg2_sb", bufs=4) as hgp2,
                tc.tile_pool(name="accD_ps", bufs=2, space="PSUM") as aps,
              ):
                for t in range(T if "D" in phases else 0):
                    hg2 = hgp2.tile([P, NG, ROW2], BF, tag="hg2")
                    for ci, c0 in enumerate(range(0, NG, 4)):
                        gch = min(4, NG - c0)
                        nc.gpsimd.dma_gather(
                            out_ap=hg2[:, c0 : c0 + gch, :],
                            in_ap=t2full[:],
                            idxs_ap=gidx2[:, t, c0 * 8 : (c0 + gch) * 8],
                            num_idxs=gch * P,
                            num_idxs_reg=gch * P,
                            elem_size=ROW2,
                            queue_num=(t * 5 + ci) % NQUEUES,
                        )
                    mp = msb.tile([P, NG, 2 * P], BF, tag="mp")
                    nc.sync.dma_start(
                        out=mp[:],
                        in_=maskw_d[:, t * NG * 2 * P : (t + 1) * NG * 2 * P],
                    )
                    ed2r = ssb.tile([P, 2], BF, tag="ed2r")
                    nc.sync.dma_start(
                        out=ed2r[:],
                        in_=t2shard[t * P : (t + 1) * P, cfg.R2_ED : cfg.R2_ED + 2],
                    )
                    ed2bf = ssb.tile([P, 1], BF, tag="ed2bf")
                    nc.scalar.activation(
                        out=ed2bf[:],
                        in_=ed2r[:].bitcast(F32),
                        func=mybir.ActivationFunctionType.Copy,
                    )
                    ep2 = aps.tile([P, NG], F32, tag="ep2")
                    for g in range(NG):
                        nc.tensor.matmul(
                            out=ep2[:, g : g + 1],
                            lhsT=mp[:, g, P : 2 * P],
                            rhs=ed2bf[:],
                            start=True, stop=True,
                        )
                    elog2 = ssb.tile([P, NG, 1], F32, tag="elog2")
                    nc.vector.tensor_tensor(
                        out=elog2[:],
                        in0=hg2[:, 0:NG, cfg.R2_ES : cfg.R2_ES + 2].bitcast(F32),
                        in1=ep2[:].rearrange("p (g h) -> p g h", h=1),
                        op=mybir.AluOpType.add,
                    )
                    e21 = ssb.tile([P, NG, 1], BF, tag="e21")
                    nc.scalar.activation(
                        out=e21[:], in_=elog2[:],
                        func=mybir.ActivationFunctionType.Exp,
                    )
                    e22 = ssb.tile([P, NG, 1], BF, tag="e22")
                    nc.scalar.activation(
                        out=e22[:], in_=elog2[:],
                        func=mybir.ActivationFunctionType.Exp, scale=cfg.NEG,
                    )
                    p2bf = ssb.tile([P, NG, 1], BF, tag="p2bf")
                    nc.vector.tensor_tensor(
                        out=p2bf[:], in0=e21[:], in1=e22[:],
                        op=mybir.AluOpType.max,
                    )
                    ph2 = ssb.tile([P, NG, OC + 1], BF, tag="ph2")
                    nc.vector.tensor_tensor(
                        out=ph2[:],
                        in0=hg2[:, 0:NG, 0 : OC + 1],
                        in1=p2bf[:, :, 0].to_broadcast([P, NG, OC + 1]),
                        op=mybir.AluOpType.mult,
                    )
                    ps3 = aps.tile([P, OC + 1], F32, tag="ps3")
                    for g in range(NG):
                        nc.tensor.matmul(
                            out=ps3[:], lhsT=mp[:, g, 0:P], rhs=ph2[:, g, :],
                            start=(g == 0), stop=(g == NG - 1),
                        )
                    rc2 = ssb.tile([P, 1], F32, tag="rc2")
                    nc.vector.reciprocal(out=rc2[:], in_=ps3[:, OC : OC + 1])
                    outsb = ssb.tile([P, OC], F32, tag="outsb")
                    nc.vector.tensor_tensor(
                        out=outsb[:],
                        in0=ps3[:, 0:OC],
                        in1=rc2[:].to_broadcast([P, OC]),
                        op=mybir.AluOpType.mult,
                    )
                    if meta["b2_nonzero"]:
                        nc.vector.tensor_tensor(
                            out=outsb[:], in0=outsb[:], in1=b2_sb[:],
                            op=mybir.AluOpType.add,
                        )
                    nc.sync.dma_start(
                        out=out_d[t * P : (t + 1) * P, :], in_=outsb[:]
                    )

    nc.compile()
    return nc


def _default_cfg(n=25000, in_ch=256, hid=128, heads=4, out_ch=64, ng=None,
                 edge_index=None):
    if ng is None:
        N = n
        ei = np.asarray(edge_index, np.int64)
        dst = np.concatenate([ei[1], np.arange(N, dtype=np.int64)])
        counts = np.bincount(dst // P, minlength=-(-n // (P * NCORES)) * NCORES)
        ng = int(np.ceil(counts.max() / P))
    return GATConfig(n, in_ch, hid, heads, out_ch, 0.2, ng)


def run(cfg, inputs, trace=False, tmpdir=None):
    in_maps, meta = host_prep(
        cfg,
        inputs["x"], inputs["edge_index"],
        inputs["W1"], inputs["a_src1"], inputs["a_dst1"], inputs["b1"],
        inputs["W2"], inputs["a_src2"], inputs["a_dst2"], inputs["b2"],
    )
    if meta["b1_nonzero"]:
        for m in in_maps:
            m["b1"] = np.tile(meta["b1"].reshape(1, -1), (P, 1))
    if meta["b2_nonzero"]:
        for m in in_maps:
            m["b2"] = np.tile(meta["b2"].reshape(1, -1), (P, 1))
    nc = build_program(cfg, meta)
    res = run_bass_kernel_spmd(
        nc,
        in_maps,
        core_ids=list(range(NCORES)),
        trace=trace,
        tmpdir=tmpdir,
    )
    shards = [res.results[c]["out"] for c in range(NCORES)]
    full = np.concatenate(shards, axis=0)[: cfg.N]
    return full, res


def kernel(**inputs):
    cfg = _default_cfg(
        n=inputs["x"].shape[0],
        in_ch=inputs["x"].shape[1],
        hid=inputs["a_src1"].shape[1],
        heads=inputs["a_src1"].shape[0],
        out_ch=inputs["a_src2"].shape[1],
        edge_index=inputs["edge_index"],
    )
    out, _ = run(cfg, inputs)
    return out.astype(np.float32)


# revision 7
# speedup vs baseline: 1.0528x; 1.0142x over previous
"""Two-layer GAT (PyG GATConv semantics) on 8 Trainium2 NeuronCores — v2.

Strategy (graph/data parallel): dst nodes range-sharded across 8 cores;
every core redundantly computes the full layer-1 node-feature table
(h = x @ W1ext, with attention projections es/ed folded in as extra
columns); edges sorted by dst and bucketed per 128-dst tile.

v2 batches all per-edge-group elementwise work into per-tile ops
(the v1 kernel spent 1.5 ms in ~5000 tiny DVE instructions), folds the
dst-row fetch into the main gather, loads each tile's one-hot masks in
a single DMA, and pipelines the layer-2 table AllGather in 5-tile
chunks under the layer-1 compute.

Row table layouts (bf16 elements):
  L1 row (640): [h0 h1 (256) | 1 1 (2) | h2 h3 (256) | 1 1 (2) |
                 es f32 x4 (516:524) | ed f32 x4 (524:532) | pad]
  The ones columns ride along in the weighted-feature matmul so a single
  pair of one-hot matmuls yields both the attention-weighted sums and the
  softmax denominators.
  L2 row (128): [h2w2 (64) | 1 (1) | pad (1) | es2 f32 (66:68) |
                 ed2 f32 (68:70) | pad]
  t2full rows are permuted: node (c,t,p) with b=t//5, q=t%5 lives at row
  b*5120 + c*640 + q*128 + p, so each 5-tile AllGather chunk lands
  contiguously.
"""

import sys

for _p in ("/opt/trn_rl_repo",):
    if _p not in sys.path:
        sys.path.insert(0, _p)

import numpy as np
import ml_dtypes

import concourse.bacc as bacc
import concourse.bass as bass
import concourse.mybir as mybir
import concourse.tile as tile
from concourse import library_config
from concourse._compat import axon_active
from concourse.bass_utils import run_bass_kernel_spmd
from concourse.masks import make_identity

BF16 = ml_dtypes.bfloat16
F32 = mybir.dt.float32
BF = mybir.dt.bfloat16
P = 128
NCORES = 8
CBOUNDS = [0, 5, 10, 15, 20, 24, 25]  # AllGather chunk tile boundaries
NQUEUES = 4  # SWDGE queues for gather desc-gen


class GATConfig:
    def __init__(self, n, in_ch, hid, heads, out_ch, neg_slope, ng):
        self.N = n
        self.NPAD = -(-n // (P * NCORES)) * (P * NCORES)
        self.SHARD = self.NPAD // NCORES
        self.T = self.SHARD // P              # dst tiles per core
        self.NT = self.NPAD // P              # node tiles (phase A)
        self.IN_CH = in_ch
        self.KIN = in_ch // P
        self.HID = hid                        # 128
        self.HEADS = heads                    # 4
        self.OUT_CH = out_ch                  # 64
        self.NEG = neg_slope
        self.NG = ng                          # edge groups per dst tile
        self.H1 = heads * hid                 # 512
        # L1 row layout (see module docstring)
        self.R1_ES = 516
        self.R1_ED = 524
        self.ROW1 = 640
        # L2 row layout
        self.R2_ES = out_ch + 2               # 66
        self.R2_ED = out_ch + 4               # 68
        self.ROW2 = 128
        self.KH1 = self.H1 // P               # 4
        assert heads == 4 and hid == 128 and in_ch % P == 0


def _wrap_idx(flat):
    """int16 flat index list -> [128, len/16] wrapped layout for dma_gather."""
    n = len(flat)
    assert n % 16 == 0
    w = np.asarray(flat, np.int16).reshape(n // 16, 16).T  # [16, n/16]
    return np.tile(w, (8, 1))                              # [128, n/16]


def _t2row(node, T):
    """t2full row index for a global node id (chunked-allgather layout)."""
    node = np.asarray(node)
    c = node // (T * P)
    r = node % (T * P)
    t = r // P
    p = r % P
    cb = np.asarray(CBOUNDS)
    k = np.searchsorted(cb, t, side="right") - 1
    w = (cb[k + 1] - cb[k]) * P
    return cb[k] * P * NCORES + c * w + (t - cb[k]) * P + p


def host_prep(cfg, x, edge_index, W1, a_src1, a_dst1, b1, W2, a_src2, a_dst2, b2):
    """Build all per-core input arrays. Returns (in_maps, meta)."""
    N, NPAD = cfg.N, cfg.NPAD
    H, C, OC = cfg.HEADS, cfg.HID, cfg.OUT_CH
    T, NG = cfg.T, cfg.NG

    # --- weights: W1 natural + es/ed projection columns appended --------
    W1 = np.asarray(W1, np.float32)
    W2 = np.asarray(W2, np.float32)
    a_src1 = np.asarray(a_src1, np.float32)
    a_dst1 = np.asarray(a_dst1, np.float32)
    a_src2 = np.asarray(a_src2, np.float32)
    a_dst2 = np.asarray(a_dst2, np.float32)
    w1ext = np.zeros((cfg.IN_CH, cfg.H1 + 8), np.float32)
    w1ext[:, : cfg.H1] = W1
    for h in range(H):
        w1ext[:, cfg.H1 + h] = W1[:, h * C : (h + 1) * C] @ a_src1[h]
        w1ext[:, cfg.H1 + 4 + h] = W1[:, h * C : (h + 1) * C] @ a_dst1[h]
    w1eh = np.ascontiguousarray(
        w1ext.reshape(cfg.KIN, P, cfg.H1 + 8).transpose(1, 0, 2)
    ).astype(BF16)                                        # [128, KIN, 520]

    w2ext = np.zeros((cfg.H1, OC + 2), np.float32)
    w2ext[:, :OC] = W2
    w2ext[:, OC] = W2 @ a_src2[0]
    w2ext[:, OC + 1] = W2 @ a_dst2[0]
    w2eh = np.ascontiguousarray(
        w2ext.reshape(cfg.KH1, P, OC + 2).transpose(1, 0, 2)
    ).astype(BF16)                                        # [128, KH1, 66]

    # --- x, transposed+tiled for lhsT ------------------------------------
    xp = np.zeros((NPAD, cfg.IN_CH), np.float32)
    xp[:N] = np.asarray(x, np.float32)
    xth = np.ascontiguousarray(
        xp.reshape(cfg.NT, P, cfg.KIN, P).transpose(3, 0, 2, 1)
    ).reshape(P, cfg.NT * cfg.KIN * P).astype(BF16)

    # --- edges ------------------------------------------------------------
    ei = np.asarray(edge_index, np.int64)
    loop = np.arange(N, dtype=np.int64)
    src = np.concatenate([ei[0], loop])
    dst = np.concatenate([ei[1], loop])
    order = np.argsort(dst, kind="stable")
    src_s = src[order].astype(np.int32)
    dst_s = dst[order].astype(np.int32)
    gtiles = cfg.NT
    counts = np.bincount(dst_s // P, minlength=gtiles)
    ng = int(np.ceil(counts.max() / P)) if counts.max() else 1
    assert ng <= NG, f"data needs NG={ng} > configured {NG}"
    ET = NG * P
    starts = np.concatenate([[0], np.cumsum(counts)])

    t2map = _t2row(np.arange(NPAD), T)                    # node -> t2full row

    gidx = np.zeros((NCORES, T, P, (NG + 1) * 8), np.int16)
    gidx2 = np.zeros((NCORES, T, P, (NG + 1) * 8), np.int16)
    maskw = np.zeros((NCORES, P, T, NG, 2 * P), BF16)
    eye = np.arange(P, dtype=np.int32)
    for gt in range(gtiles):
        c, t = divmod(gt, T)
        lo, hi = starts[gt], starts[gt + 1]
        k = hi - lo
        idx = np.zeros(ET + P, np.int32)
        idx[:k] = src_s[lo:hi]
        idx[ET : ET + P] = gt * P + eye                   # own dst rows
        dl = np.full(ET, -1, np.int32)
        dl[:k] = dst_s[lo:hi] - gt * P
        # pad dst nodes get a dummy self-edge so their softmax denom is > 0
        pads = eye[gt * P + eye >= N]
        assert k + len(pads) <= ET
        idx[k : k + len(pads)] = gt * P + pads
        dl[k : k + len(pads)] = pads
        gidx[c, t] = _wrap_idx(idx.astype(np.int16))
        gidx2[c, t, :, : NG * 8] = _wrap_idx(t2map[idx[:ET]].astype(np.int16))
        m = dl.reshape(NG, P, 1) == eye.reshape(1, 1, P)  # [NG, e, d]
        maskw[c, :, t, :, :P] = m.transpose(1, 0, 2)      # [e, NG, d]
        maskw[c, :, t, :, P:] = m.transpose(2, 0, 1)      # [d, NG, e]

    in_maps = []
    for c in range(NCORES):
        in_maps.append(
            {
                "xth": xth,
                "w1eh": w1eh,
                "w2eh": w2eh,
                "gidx": gidx[c].reshape(T * P, (NG + 1) * 8),
                "gidx2": gidx2[c].reshape(T * P, (NG + 1) * 8),
                "maskw": maskw[c].reshape(P, T * NG * 2 * P),
            }
        )
    meta = {
        "b1_nonzero": bool(np.any(np.asarray(b1))),
        "b2_nonzero": bool(np.any(np.asarray(b2))),
        "b1": np.asarray(b1, np.float32),
        "b2": np.asarray(b2, np.float32),
    }
    return in_maps, meta


def build_program(cfg, meta, phases="ABCD", chunked_cc=True):
    under_axon = axon_active()
    nc = bacc.Bacc(
        "TRN2",
        target_bir_lowering=False,
        debug=not under_axon,
        num_devices=NCORES,
        dynamic_dma_scratch_size=65536,
        num_swdge_queues=NQUEUES,
    )
    H, C, OC, NG, T = cfg.HEADS, cfg.HID, cfg.OUT_CH, cfg.NG, cfg.T
    H1, KIN, KH1 = cfg.H1, cfg.KIN, cfg.KH1
    ROW1, ROW2 = cfg.ROW1, cfg.ROW2
    ACHUNK = 25                               # phase-A node tiles per x chunk

    xth_d = nc.dram_tensor("xth", [P, cfg.NT * KIN * P], BF, kind="ExternalInput")
    w1eh_d = nc.dram_tensor("w1eh", [P, KIN, H1 + 8], BF, kind="ExternalInput")
    w2eh_d = nc.dram_tensor("w2eh", [P, KH1, OC + 2], BF, kind="ExternalInput")
    gidx_d = nc.dram_tensor("gidx", [T * P, (NG + 1) * 8], mybir.dt.int16,
                            kind="ExternalInput")
    gidx2_d = nc.dram_tensor("gidx2", [T * P, (NG + 1) * 8], mybir.dt.int16,
                             kind="ExternalInput")
    maskw_d = nc.dram_tensor("maskw", [P, T * NG * 2 * P], BF,
                             kind="ExternalInput")
    out_d = nc.dram_tensor("out", [cfg.SHARD, OC], F32, kind="ExternalOutput")

    table1 = nc.dram_tensor("table1", [cfg.NPAD, ROW1], BF)
    t2shard = nc.dram_tensor("t2shard", [cfg.SHARD, ROW2], BF)
    t2full = nc.dram_tensor("t2full", [cfg.NPAD, ROW2], BF, addr_space="Shared")

    if meta["b1_nonzero"]:
        b1_d = nc.dram_tensor("b1", [P, H1], F32, kind="ExternalInput")
    if meta["b2_nonzero"]:
        b2_d = nc.dram_tensor("b2", [P, OC], F32, kind="ExternalInput")

    with tile.TileContext(nc) as tc:
        nc.gpsimd.load_library(library_config.mlp)

        with tc.tile_pool(name="persist", bufs=1) as pp:
            w1eh = pp.tile([P, KIN, H1 + 8], BF)
            nc.sync.dma_start(out=w1eh[:], in_=w1eh_d[:])
            w2eh = pp.tile([P, KH1, OC + 2], BF)
            nc.sync.dma_start(out=w2eh[:], in_=w2eh_d[:])
            gidx = pp.tile([P, T, (NG + 1) * 8], mybir.dt.int16)
            nc.sync.dma_start(
                out=gidx[:], in_=gidx_d[:].rearrange("(t p) s -> p t s", p=P)
            )
            gidx2 = pp.tile([P, T, (NG + 1) * 8], mybir.dt.int16)
            nc.sync.dma_start(
                out=gidx2[:], in_=gidx2_d[:].rearrange("(t p) s -> p t s", p=P)
            )
            ident = pp.tile([P, P], BF)
            make_identity(nc, ident[:])
            if meta["b1_nonzero"]:
                b1_sb = pp.tile([P, H1], F32)
                nc.sync.dma_start(out=b1_sb[:], in_=b1_d[:])
            if meta["b2_nonzero"]:
                b2_sb = pp.tile([P, OC], F32)
                nc.sync.dma_start(out=b2_sb[:], in_=b2_d[:])

            # persistent stage buffers with ones/pad pre-set
            NSTAGE = 3
            stages = []
            for i in range(NSTAGE):
                s = pp.tile([P, 2, ROW1], BF, name=f"stage{i}")
                nc.vector.memset(s[:, :, 256:258], 1.0)
                nc.vector.memset(s[:, :, 514:516], 1.0)
                nc.vector.memset(s[:, :, 532:ROW1], 0.0)
                stages.append(s)
            stages2 = []
            for i in range(NSTAGE):
                s = pp.tile([P, ROW2], BF, name=f"stage2_{i}")
                nc.vector.memset(s[:, OC : OC + 2], 0.0)
                nc.vector.memset(s[:, OC : OC + 1], 1.0)
                nc.vector.memset(s[:, cfg.R2_ED + 2 :], 0.0)
                stages2.append(s)

            # ---------------- Phase A: h table ---------------------------
            if "A" not in phases:
                raise ValueError("phase A required")
            with (
                tc.tile_pool(name="xc_pool", bufs=2) as xcp,
                tc.tile_pool(name="pa_ps", bufs=4, space="PSUM") as pa_ps,
            ):
                for c0 in range(0, cfg.NT, ACHUNK):
                    xc = xcp.tile([P, ACHUNK * KIN * P], BF, tag="xc")
                    nc.sync.dma_start(
                        out=xc[:],
                        in_=xth_d[:, c0 * KIN * P : (c0 + ACHUNK) * KIN * P],
                    )
                    for i in range(ACHUNK):
                        nt = c0 + i
                        ps01 = pa_ps.tile([P, 256], F32, tag="ps01")
                        ps23 = pa_ps.tile([P, 264], F32, tag="ps23")
                        for j in range(KIN):
                            lhs = xc[:, (i * KIN + j) * P : (i * KIN + j + 1) * P]
                            nc.tensor.matmul(
                                out=ps01[:], lhsT=lhs, rhs=w1eh[:, j, 0:256],
                                start=(j == 0), stop=(j == KIN - 1),
                            )
                            nc.tensor.matmul(
                                out=ps23[:], lhsT=lhs, rhs=w1eh[:, j, 256:520],
                                start=(j == 0), stop=(j == KIN - 1),
                            )
                        stage = stages[(nt // 2) % NSTAGE]
                        half = nt % 2
                        nc.scalar.activation(
                            out=stage[:, half, 0:256], in_=ps01[:],
                            func=mybir.ActivationFunctionType.Copy,
                        )
                        nc.vector.tensor_copy(
                            out=stage[:, half, 258:514], in_=ps23[:, 0:256]
                        )
                        nc.scalar.activation(
                            out=stage[:, half, 516:532].bitcast(F32),
                            in_=ps23[:, 256:264],
                            func=mybir.ActivationFunctionType.Copy,
                        )
                        if half == 1:
                            nc.sync.dma_start(
                                out=table1[(nt - 1) * P : (nt + 1) * P, :].rearrange(
                                    "(t p) r -> p t r", p=P
                                ),
                                in_=stage[:],
                            )

            # ---------------- Phases B/C/D --------------------------------
            with (
                tc.tile_pool(name="mask_sb", bufs=2) as msb,
                tc.tile_pool(name="small_sb", bufs=3) as ssb,
            ):
              with (
                tc.tile_pool(name="hg_sb", bufs=2) as hgp,
                tc.tile_pool(name="hg2_sb", bufs=4) as hgp2,
                tc.tile_pool(name="ph_sb", bufs=2) as php,
                tc.tile_pool(name="elu_sb", bufs=1) as elup,
                tc.tile_pool(name="acc_ps", bufs=2, space="PSUM") as aps,
                tc.tile_pool(name="tp_ps", bufs=1, space="PSUM") as tps,
              ):
                # ---- layer 1 aggregation + table2 rows + chunked gather --
                for t in range(T if "B" in phases else 0):
                    hg = hgp.tile([P, NG + 1, ROW1], BF, tag="hg")
                    for ci, c0 in enumerate(range(0, NG + 1, 4)):
                        gch = min(4, NG + 1 - c0)
                        nc.gpsimd.dma_gather(
                            out_ap=hg[:, c0 : c0 + gch, :],
                            in_ap=table1[:],
                            idxs_ap=gidx[:, t, c0 * 8 : (c0 + gch) * 8],
                            num_idxs=gch * P,
                            num_idxs_reg=gch * P,
                            elem_size=ROW1,
                            queue_num=(t * 5 + ci) % NQUEUES,
                        )
                    mp = msb.tile([P, NG, 2 * P], BF, tag="mp")
                    nc.sync.dma_start(
                        out=mp[:],
                        in_=maskw_d[:, t * NG * 2 * P : (t + 1) * NG * 2 * P],
                    )
                    # ed for this tile's dsts (from the appended dst rows)
                    edbf = ssb.tile([P, H], BF, tag="edbf")
                    nc.scalar.activation(
                        out=edbf[:],
                        in_=hg[:, NG, cfg.R1_ED : cfg.R1_ED + 8].bitcast(F32),
                        func=mybir.ActivationFunctionType.Copy,
                    )
                    # ed scattered to edge slots: one psum, NG matmuls
                    ep = aps.tile([P, NG * H], F32, tag="ep")
                    for g in range(NG):
                        nc.tensor.matmul(
                            out=ep[:, g * H : (g + 1) * H],
                            lhsT=mp[:, g, P : 2 * P],
                            rhs=edbf[:],
                            start=True, stop=True,
                        )
                    # batched logits chain
                    elog = ssb.tile([P, NG, H], F32, tag="elog")
                    nc.vector.tensor_tensor(
                        out=elog[:],
                        in0=hg[:, 0:NG, cfg.R1_ES : cfg.R1_ES + 8].bitcast(F32),
                        in1=ep[:].rearrange("p (g h) -> p g h", h=H),
                        op=mybir.AluOpType.add,
                    )
                    e1 = ssb.tile([P, NG, H], BF, tag="e1")
                    nc.scalar.activation(
                        out=e1[:], in_=elog[:],
                        func=mybir.ActivationFunctionType.Exp,
                    )
                    e2 = ssb.tile([P, NG, H], BF, tag="e2")
                    nc.scalar.activation(
                        out=e2[:], in_=elog[:],
                        func=mybir.ActivationFunctionType.Exp, scale=cfg.NEG,
                    )
                    # p = exp(lrelu(x)) = max(exp(x), exp(0.2x)); write the max
                    # straight into ph's ones columns
                    ph = php.tile([P, NG, 516], BF, tag="ph")
                    nc.vector.tensor_tensor(
                        out=ph[:, :, 256:258], in0=e1[:, :, 0:2], in1=e2[:, :, 0:2],
                        op=mybir.AluOpType.max,
                    )
                    nc.vector.tensor_tensor(
                        out=ph[:, :, 514:516], in0=e1[:, :, 2:4], in1=e2[:, :, 2:4],
                        op=mybir.AluOpType.max,
                    )
                    for h0, (pc, hc) in enumerate(
                        ((256, 0), (257, 128), (514, 258), (515, 386))
                    ):
                        nc.vector.tensor_tensor(
                            out=ph[:, :, hc : hc + C],
                            in0=hg[:, 0:NG, hc : hc + C],
                            in1=ph[:, :, pc : pc + 1].to_broadcast([P, NG, C]),
                            op=mybir.AluOpType.mult,
                        )
                    # aggregate to dsts
                    ps1 = aps.tile([P, 258], F32, tag="ps1")
                    ps2 = aps.tile([P, 258], F32, tag="ps2")
                    for g in range(NG):
                        nc.tensor.matmul(
                            out=ps1[:], lhsT=mp[:, g, 0:P], rhs=ph[:, g, 0:258],
                            start=(g == 0), stop=(g == NG - 1),
                        )
                        nc.tensor.matmul(
                            out=ps2[:], lhsT=mp[:, g, 0:P], rhs=ph[:, g, 258:516],
                            start=(g == 0), stop=(g == NG - 1),
                        )
                    # softmax denominators -> reciprocal
                    rc = ssb.tile([P, H], F32, tag="rc")
                    nc.vector.reciprocal(out=rc[:, 0:2], in_=ps1[:, 256:258])
                    nc.vector.reciprocal(out=rc[:, 2:4], in_=ps2[:, 256:258])
                    v = elup.tile([P, H1], F32, tag="v")
                    nc.vector.tensor_tensor(
                        out=v[:, 0:256].rearrange("p (h c) -> p h c", c=C),
                        in0=ps1[:, 0:256].rearrange("p (h c) -> p h c", c=C),
                        in1=rc[:, 0:2].to_broadcast([P, 2, C]),
                        op=mybir.AluOpType.mult,
                    )
                    nc.vector.tensor_tensor(
                        out=v[:, 256:512].rearrange("p (h c) -> p h c", c=C),
                        in0=ps2[:, 0:256].rearrange("p (h c) -> p h c", c=C),
                        in1=rc[:, 2:4].to_broadcast([P, 2, C]),
                        op=mybir.AluOpType.mult,
                    )
                    if meta["b1_nonzero"]:
                        nc.vector.tensor_tensor(
                            out=v[:], in0=v[:], in1=b1_sb[:],
                            op=mybir.AluOpType.add,
                        )
                    # ELU -> bf16:  elu(v) = relu(v) + exp(-relu(-v)) - 1
                    rneg = elup.tile([P, H1], F32, tag="rneg")
                    nc.scalar.activation(
                        out=rneg[:], in_=v[:],
                        func=mybir.ActivationFunctionType.Relu, scale=-1.0,
                    )
                    sexp = elup.tile([P, H1], F32, tag="sexp")
                    nc.scalar.activation(
                        out=sexp[:], in_=rneg[:],
                        func=mybir.ActivationFunctionType.Exp, scale=-1.0,
                    )
                    rpos = elup.tile([P, H1], F32, tag="rpos")
                    nc.scalar.activation(
                        out=rpos[:], in_=v[:],
                        func=mybir.ActivationFunctionType.Relu,
                    )
                    nc.vector.tensor_tensor(
                        out=sexp[:], in0=rpos[:], in1=sexp[:],
                        op=mybir.AluOpType.add,
                    )
                    h2bf = elup.tile([P, H1], BF, tag="h2bf")
                    nc.scalar.activation(
                        out=h2bf[:], in_=sexp[:],
                        func=mybir.ActivationFunctionType.Copy, bias=-1.0,
                    )
                    # transpose h2, W2ext matmul
                    h2p = tps.tile([P, OC + 2], F32, tag="h2p")
                    for j in range(KH1):
                        tp = tps.tile([P, P], BF, tag="tp")
                        nc.tensor.transpose(
                            out=tp[:], in_=h2bf[:, j * P : (j + 1) * P],
                            identity=ident[:],
                        )
                        h2t = ssb.tile([P, P], BF, tag="h2t")
                        nc.scalar.activation(
                            out=h2t[:], in_=tp[:],
                            func=mybir.ActivationFunctionType.Copy,
                        )
                        nc.tensor.matmul(
                            out=h2p[:], lhsT=h2t[:], rhs=w2eh[:, j, :],
                            start=(j == 0), stop=(j == KH1 - 1),
                        )
                    stage2 = stages2[t % NSTAGE]
                    nc.scalar.activation(
                        out=stage2[:, 0:OC], in_=h2p[:, 0:OC],
                        func=mybir.ActivationFunctionType.Copy,
                    )
                    nc.vector.tensor_copy(
                        out=stage2[:, cfg.R2_ES : cfg.R2_ES + 4].bitcast(F32),
                        in_=h2p[:, OC : OC + 2],
                    )
                    nc.sync.dma_start(
                        out=t2shard[t * P : (t + 1) * P, :], in_=stage2[:]
                    )
                    # chunked allgather as soon as a chunk's tiles are done
                    if "C" in phases and chunked_cc and (t + 1) in CBOUNDS:
                        k = CBOUNDS.index(t + 1) - 1
                        lo, hi = CBOUNDS[k], CBOUNDS[k + 1]
                        nc.gpsimd.collective_compute(
                            "AllGather",
                            mybir.AluOpType.bypass,
                            replica_groups=[list(range(NCORES))],
                            ins=[t2shard[lo * P : hi * P, :].opt()],
                            outs=[
                                t2full[lo * P * NCORES : hi * P * NCORES, :].opt()
                            ],
                        )

              if "C" in phases and not chunked_cc:
                for k in range(len(CBOUNDS) - 1):
                    lo, hi = CBOUNDS[k], CBOUNDS[k + 1]
                    nc.gpsimd.collective_compute(
                        "AllGather",
                        mybir.AluOpType.bypass,
                        replica_groups=[list(range(NCORES))],
                        ins=[t2shard[lo * P : hi * P, :].opt()],
                        outs=[t2full[lo * P * NCORES : hi * P * NCORES, :].opt()],
                    )

              # ---- layer 2 aggregation -> output ----
              with (
                tc.tile_pool(name="hg2_sb", bufs=4) as hgp2,
                tc.tile_pool(name="accD_ps", bufs=2, space="PSUM") as aps,
              ):
                for t in range(T if "D" in phases else 0):
                    hg2 = hgp2.tile([P, NG, ROW2], BF, tag="hg2")
                    for ci, c0 in enumerate(range(0, NG, 4)):
                        gch = min(4, NG - c0)
                        nc.gpsimd.dma_gather(
                            out_ap=hg2[:, c0 : c0 + gch, :],
                            in_ap=t2full[:],
                            idxs_ap=gidx2[:, t, c0 * 8 : (c0 + gch) * 8],
                            num_idxs=gch * P,
                            num_idxs_reg=gch * P,
                            elem_size=ROW2,
                            queue_num=(t * 5 + ci) % NQUEUES,
                        )
                    mp = msb.tile([P, NG, 2 * P], BF, tag="mp")
                    nc.sync.dma_start(
                        out=mp[:],
                        in_=maskw_d[:, t * NG * 2 * P : (t + 1) * NG * 2 * P],
                    )
                    ed2r = ssb.tile([P, 2], BF, tag="ed2r")
                    nc.sync.dma_start(
                        out=ed2r[:],
                        in_=t2shard[t * P : (t + 1) * P, cfg.R2_ED : cfg.R2_ED + 2],
                    )
                    ed2bf = ssb.tile([P, 1], BF, tag="ed2bf")
                    nc.scalar.activation(
                        out=ed2bf[:],
                        in_=ed2r[:].bitcast(F32),
                        func=mybir.ActivationFunctionType.Copy,
                    )
                    ep2 = aps.tile([P, NG], F32, tag="ep2")
                    for g in range(NG):
                        nc.tensor.matmul(
                            out=ep2[:, g : g + 1],
                            lhsT=mp[:, g, P : 2 * P],
                            rhs=ed2bf[:],
                            start=True, stop=True,
                        )
                    elog2 = ssb.tile([P, NG, 1], F32, tag="elog2")
                    nc.vector.tensor_tensor(
                        out=elog2[:],
                        in0=hg2[:, 0:NG, cfg.R2_ES : cfg.R2_ES + 2].bitcast(F32),
                        in1=ep2[:].rearrange("p (g h) -> p g h", h=1),
                        op=mybir.AluOpType.add,
                    )
                    e21 = ssb.tile([P, NG, 1], BF, tag="e21")
                    nc.scalar.activation(
                        out=e21[:], in_=elog2[:],
                        func=mybir.ActivationFunctionType.Exp,
                    )
                    e22 = ssb.tile([P, NG, 1], BF, tag="e22")
                    nc.scalar.activation(
                        out=e22[:], in_=elog2[:],
                        func=mybir.ActivationFunctionType.Exp, scale=cfg.NEG,
                    )
                    p2bf = ssb.tile([P, NG, 1], BF, tag="p2bf")
                    nc.vector.tensor_tensor(
                        out=p2bf[:], in0=e21[:], in1=e22[:],
                        op=mybir.AluOpType.max,
                    )
                    ph2 = ssb.tile([P, NG, OC + 1], BF, tag="ph2")
                    nc.vector.tensor_tensor(
                        out=ph2[:],
                        in0=hg2[:, 0:NG, 0 : OC + 1],
                        in1=p2bf[:, :, 0].to_broadcast([P, NG, OC + 1]),
                        op=mybir.AluOpType.mult,
                    )
                    ps3 = aps.tile([P, OC + 1], F32, tag="ps3")
                    for g in range(NG):
                        nc.tensor.matmul(
                            out=ps3[:], lhsT=mp[:, g, 0:P], rhs=ph2[:, g, :],
                            start=(g == 0), stop=(g == NG - 1),
                        )
                    rc2 = ssb.tile([P, 1], F32, tag="rc2")
                    nc.vector.reciprocal(out=rc2[:], in_=ps3[:, OC : OC + 1])
                    outsb = ssb.tile([P, OC], F32, tag="outsb")
                    nc.vector.tensor_tensor(
                        out=outsb[:],
                        in0=ps3[:, 0:OC],
                        in1=rc2[:].to_broadcast([P, OC]),
                        op=mybir.AluOpType.mult,
                    )
                    if meta["b2_nonzero"]:
                        nc.vector.tensor_tensor(
                            out=outsb[:], in0=outsb[:], in1=b2_sb[:],
                            op=mybir.AluOpType.add,
                        )
                    nc.sync.dma_start(
                        out=out_d[t * P : (t + 1) * P, :], in_=outsb[:]
                    )

    nc.compile()
    return nc


def _default_cfg(n=25000, in_ch=256, hid=128, heads=4, out_ch=64, ng=None,
                 edge_index=None):
    if ng is None:
        N = n
        ei = np.asarray(edge_index, np.int64)
        dst = np.concatenate([ei[1], np.arange(N, dtype=np.int64)])
        counts = np.bincount(dst // P, minlength=-(-n // (P * NCORES)) * NCORES)
        ng = int(np.ceil(counts.max() / P))
    return GATConfig(n, in_ch, hid, heads, out_ch, 0.2, ng)


def run(cfg, inputs, trace=False, tmpdir=None):
    in_maps, meta = host_prep(
        cfg,
        inputs["x"], inputs["edge_index"],
        inputs["W1"], inputs["a_src1"], inputs["a_dst1"], inputs["b1"],
        inputs["W2"], inputs["a_src2"], inputs["a_dst2"], inputs["b2"],
    )
    if meta["b1_nonzero"]:
        for m in in_maps:
            m["b1"] = np.tile(meta["b1"].reshape(1, -1), (P, 1))
    if meta["b2_nonzero"]:
        for m in in_maps:
            m["b2"] = np.tile(meta["b2"].reshape(1, -1), (P, 1))
    nc = build_program(cfg, meta)
    res = run_bass_kernel_spmd(
        nc,
        in_maps,
        core_ids=list(range(NCORES)),
        trace=trace,
        tmpdir=tmpdir,
    )
    shards = [res.results[c]["out"] for c in range(NCORES)]
    full = np.concatenate(shards, axis=0)[: cfg.N]
    return full, res


def kernel(**inputs):
    cfg = _default_cfg(
        n=inputs["x"].shape[0],
        in_ch=inputs["x"].shape[1],
        hid=inputs["a_src1"].shape[1],
        heads=inputs["a_src1"].shape[0],
        out_ch=inputs["a_src2"].shape[1],
        edge_index=inputs["edge_index"],
    )
    out, _ = run(cfg, inputs)
    return out.astype(np.float32)
